# revision 1
# baseline (speedup 1.0000x reference)
"""Trainium2 Bass kernel for nn_Contextual_MFN (Memory Fusion Network).

Structure (per core; batch data-parallel 8 ways, 32 rows/core):
  phase 0: xWb[t] = Wih_aug @ x_aug[t]  (all t, fp32r matmuls, bias folded in)
  phase 1: sequential 3xLSTM recurrence; gates = xWb (identity-inject) + Whh@h
  phase 2a: time-parallel attention: att1 MLP -> exp -> U = E*cStar (unnormalized),
            S = sum(E), att2/g1/g2 linear parts on U, bias*S folds
  recip:   Sinv = 1/S
  phase 3: sequential memory-gate recurrence (mem-dependent matmuls only)
  phase 4: output MLP on [h_l, h_a, h_v, mem]

All activations feature-major: [features(partitions), batch(free)].
"""
import os
import numpy as np

import concourse.bass as bass
import concourse.tile as tile
from concourse import bacc, mybir
from concourse.bass_utils import run_bass_kernel_spmd

F32 = mybir.dt.float32
USE_F32R = True
F32R = mybir.dt.float32r
AF = mybir.ActivationFunctionType

# Problem constants (hardcoded; kernel.py must be self-contained)
T_FULL = 512
NBATCH = 256
NCORES = 8
B = NBATCH // NCORES          # 32 batch rows per core
D_L, D_A, D_V = 300, 74, 35
DIN = D_L + D_A + D_V         # 409
DAUG = DIN + 1                # 410 (ones row for bias)
DH = 128
MEM = 256
CH0 = 16                      # phase-0 chunk (steps)
CH2 = 8                       # phase-2a / phase-3 chunk (steps)

# gate slot order: s = g'*3 + m, with g' in (i, f, o, g_tanh); torch rows are (i, f, g, o)
TORCH_G = (0, 1, 3, 2)        # our slot g' -> torch gate row block


def _nonzero_kcs(s):
    """Phase-0 K-chunks (of Waug rows 0..409 padded to 512) that are nonzero for
    output slot s. m=0 (l): feats 0-299 -> kc 0,1,2 (+ones kc3). m=1 (a): 300-373
    -> kc2 (+kc3 ones). m=2 (v): 374-408 -> kc2,kc3 (+ones kc3)."""
    m = s % 3
    if m == 0:
        return [0, 1, 2, 3]
    return [2, 3]


def build_program(Tp=T_FULL):
    global F32R
    F32R = mybir.dt.float32r if USE_F32R else F32
    assert Tp % CH0 == 0 and Tp % CH2 == 0
    NCH2 = Tp // CH2
    nc = bacc.Bacc("TRN2", target_bir_lowering=False, debug=False)

    # ---------------- external inputs ----------------
    xT = nc.dram_tensor("xT", [DAUG, Tp * B], F32, kind="ExternalInput")
    waug = nc.dram_tensor("waug", [512, 1536], F32, kind="ExternalInput")
    whhT = nc.dram_tensor("whhT", [128, 1536], F32, kind="ExternalInput")
    ident = nc.dram_tensor("ident", [128, 128], F32, kind="ExternalInput")
    ones128 = nc.dram_tensor("ones128", [128, 1], F32, kind="ExternalInput")

    a1w1 = nc.dram_tensor("a1w1", [768, 256], F32, kind="ExternalInput")
    a1b1 = nc.dram_tensor("a1b1", [128, 2], F32, kind="ExternalInput")
    a1w2 = nc.dram_tensor("a1w2", [256, 768], F32, kind="ExternalInput")
    a1b2 = nc.dram_tensor("a1b2", [128, 6], F32, kind="ExternalInput")
    a2w1 = nc.dram_tensor("a2w1", [768, 256], F32, kind="ExternalInput")
    a2b1r = nc.dram_tensor("a2b1r", [1, 256], F32, kind="ExternalInput")
    a2w2 = nc.dram_tensor("a2w2", [256, 256], F32, kind="ExternalInput")
    a2b2r = nc.dram_tensor("a2b2r", [1, 256], F32, kind="ExternalInput")
    g1a = nc.dram_tensor("g1a", [768, 256], F32, kind="ExternalInput")
    g2a = nc.dram_tensor("g2a", [768, 256], F32, kind="ExternalInput")
    g1b = nc.dram_tensor("g1b", [256, 256], F32, kind="ExternalInput")
    g2b = nc.dram_tensor("g2b", [256, 256], F32, kind="ExternalInput")
    g1b1r = nc.dram_tensor("g1b1r", [1, 256], F32, kind="ExternalInput")
    g2b1r = nc.dram_tensor("g2b1r", [1, 256], F32, kind="ExternalInput")
    g1w2 = nc.dram_tensor("g1w2", [256, 256], F32, kind="ExternalInput")
    g2w2 = nc.dram_tensor("g2w2", [256, 256], F32, kind="ExternalInput")
    gb2r = nc.dram_tensor("gb2r", [1, 512], F32, kind="ExternalInput")
    ow1 = nc.dram_tensor("ow1", [640, 256], F32, kind="ExternalInput")
    ob1 = nc.dram_tensor("ob1", [128, 2], F32, kind="ExternalInput")
    ow2 = nc.dram_tensor("ow2", [256, 1], F32, kind="ExternalInput")
    ob2 = nc.dram_tensor("ob2", [1, 1], F32, kind="ExternalInput")

    out_d = nc.dram_tensor("out", [B, 1], F32, kind="ExternalOutput")

    # ---------------- internal dram scratch ----------------
    xwb = nc.dram_tensor("xwb", [12, Tp, 128, B], F32)
    cs = [nc.dram_tensor(f"cseq{m}", [Tp + 1, 128, B], F32) for m in range(3)]
    a2r_d = nc.dram_tensor("a2r_d", [NCH2, 128, 2 * CH2 * B], F32)
    g1p_d = nc.dram_tensor("g1p_d", [NCH2, 128, 2 * CH2 * B], F32)
    g2p_d = nc.dram_tensor("g2p_d", [NCH2, 128, 2 * CH2 * B], F32)
    s_d = nc.dram_tensor("s_d", [NCH2, CH2 * B], F32)
    sinv_d = nc.dram_tensor("sinv_d", [NCH2, CH2 * B], F32)

    NB2 = CH2 * B  # 256: phase-2a matmul free dim

    import contextlib
    with tile.TileContext(nc) as tc:
        ctx = contextlib.ExitStack()
        with ctx:
            wpool = ctx.enter_context(tc.tile_pool(name="weights", bufs=1))
            hpool = ctx.enter_context(tc.tile_pool(name="hstate", bufs=2))

            # ---- resident weights / constants in SBUF ----
            wihT_t = wpool.tile([128, 4, 1536], F32R)
            nc.sync.dma_start(
                wihT_t[:], waug.ap().rearrange("(kc p) c -> p kc c", p=128).bitcast(F32R))
            whhT_t = wpool.tile([128, 1536], F32)
            nc.sync.dma_start(whhT_t[:], whhT.ap())
            id_t = wpool.tile([128, 128], F32R)
            nc.sync.dma_start(id_t[:], ident.ap().bitcast(F32R))
            ones128_t = wpool.tile([128, 1], F32R)
            nc.sync.dma_start(ones128_t[:], ones128.ap().bitcast(F32R))

            a1w1_t = wpool.tile([128, 6, 256], F32R)
            nc.sync.dma_start(a1w1_t[:], a1w1.ap().rearrange("(kc p) c -> p kc c", p=128).bitcast(F32R))
            a1b1_t = wpool.tile([128, 2], F32)
            nc.sync.dma_start(a1b1_t[:], a1b1.ap())
            a1w2_t = wpool.tile([128, 2, 768], F32R)
            nc.sync.dma_start(a1w2_t[:], a1w2.ap().rearrange("(kc p) c -> p kc c", p=128).bitcast(F32R))
            a1b2_t = wpool.tile([128, 6], F32)
            nc.sync.dma_start(a1b2_t[:], a1b2.ap())
            a2w1_t = wpool.tile([128, 6, 256], F32R)
            nc.sync.dma_start(a2w1_t[:], a2w1.ap().rearrange("(kc p) c -> p kc c", p=128).bitcast(F32R))
            a2b1r_t = wpool.tile([1, 256], F32R)
            nc.sync.dma_start(a2b1r_t[:], a2b1r.ap().bitcast(F32R))
            a2w2_t = wpool.tile([128, 2, 256], F32R)
            nc.sync.dma_start(a2w2_t[:], a2w2.ap().rearrange("(kc p) c -> p kc c", p=128).bitcast(F32R))
            a2b2r_t = wpool.tile([1, 256], F32R)
            nc.sync.dma_start(a2b2r_t[:], a2b2r.ap().bitcast(F32R))
            g1a_t = wpool.tile([128, 6, 256], F32R)
            nc.sync.dma_start(g1a_t[:], g1a.ap().rearrange("(kc p) c -> p kc c", p=128).bitcast(F32R))
            g2a_t = wpool.tile([128, 6, 256], F32R)
            nc.sync.dma_start(g2a_t[:], g2a.ap().rearrange("(kc p) c -> p kc c", p=128).bitcast(F32R))
            g1b1r_t = wpool.tile([1, 256], F32R)
            nc.sync.dma_start(g1b1r_t[:], g1b1r.ap().bitcast(F32R))
            g2b1r_t = wpool.tile([1, 256], F32R)
            nc.sync.dma_start(g2b1r_t[:], g2b1r.ap().bitcast(F32R))
            g1b_t = wpool.tile([128, 2, 256], F32)
            nc.sync.dma_start(g1b_t[:], g1b.ap().rearrange("(kc p) c -> p kc c", p=128))
            g2b_t = wpool.tile([128, 2, 256], F32)
            nc.sync.dma_start(g2b_t[:], g2b.ap().rearrange("(kc p) c -> p kc c", p=128))
            g1w2_t = wpool.tile([128, 2, 256], F32)
            nc.sync.dma_start(g1w2_t[:], g1w2.ap().rearrange("(kc p) c -> p kc c", p=128))
            g2w2_t = wpool.tile([128, 2, 256], F32)
            nc.sync.dma_start(g2w2_t[:], g2w2.ap().rearrange("(kc p) c -> p kc c", p=128))
            gb2r_t = wpool.tile([1, 512], F32)
            nc.sync.dma_start(gb2r_t[:], gb2r.ap())
            ow1_t = wpool.tile([128, 5, 256], F32)
            nc.sync.dma_start(ow1_t[:], ow1.ap().rearrange("(kc p) c -> p kc c", p=128))
            ob1_t = wpool.tile([128, 2], F32)
            nc.sync.dma_start(ob1_t[:], ob1.ap())
            ow2_t = wpool.tile([128, 2, 1], F32)
            nc.sync.dma_start(ow2_t[:], ow2.ap().rearrange("(kc p) c -> p kc c", p=128))
            ob2_t = wpool.tile([1, 1], F32)
            nc.sync.dma_start(ob2_t[:], ob2.ap())

            ones32_t = wpool.tile([1, 32], F32)
            nc.vector.memset(ones32_t[:], 1.0)
            ones1x128_t = wpool.tile([1, 128], F32)
            nc.vector.memset(ones1x128_t[:], 1.0)
            zero_t = wpool.tile([128, 32], F32)
            nc.vector.memset(zero_t[:], 0.0)

            # =================== PHASE 0: xWb ===================
            with (
                tc.tile_pool(name="p0x", bufs=2) as p0x,
                tc.tile_pool(name="p0s", bufs=4) as p0s,
                tc.tile_pool(name="p0p", bufs=4, space="PSUM") as p0p,
            ):
                for k0 in range(Tp // CH0):
                    t0 = k0 * CH0
                    n0 = CH0 * B  # 512
                    xt = p0x.tile([128, 4, n0], F32R, tag="xt")
                    for kc in range(4):
                        rows = 128 if kc < 3 else DAUG - 384  # 26 on last chunk
                        nc.sync.dma_start(
                            xt[0:rows, kc, :],
                            xT.ap()[kc * 128:kc * 128 + rows, t0 * B:(t0 + CH0) * B].bitcast(F32R))
                    for s in range(12):
                        pt = p0p.tile([128, n0], F32, tag="p0acc")
                        kcs = _nonzero_kcs(s)
                        for i, kc in enumerate(kcs):
                            rows = 128 if kc < 3 else DAUG - 384
                            nc.tensor.matmul(
                                pt[:], wihT_t[0:rows, kc, s * 128:(s + 1) * 128],
                                xt[0:rows, kc, :],
                                start=(i == 0), stop=(i == len(kcs) - 1))
                        st = p0s.tile([128, n0], F32, tag="p0st")
                        if s % 2 == 0:
                            nc.vector.tensor_copy(st[:], pt[:])
                        else:
                            nc.scalar.copy(st[:], pt[:])
                        # dram [CH0, 128, B] slab, partition-major write
                        nc.sync.dma_start(
                            xwb.ap()[s, t0:t0 + CH0, :, :].transpose([1, 0, 2]),
                            st[:].rearrange("p (t b) -> p t b", b=B))

            tc.strict_bb_all_engine_barrier()

            # =================== PHASE 1: LSTM recurrence ===================
            with (
                tc.tile_pool(name="p1w", bufs=2) as p1w,
                tc.tile_pool(name="p1s", bufs=3) as p1s,
                tc.tile_pool(name="p1c", bufs=4) as p1c,
                tc.tile_pool(name="p1p", bufs=2, space="PSUM") as p1p,
            ):
                h_cur = hpool.tile([128, 96], F32, tag="h")
                nc.vector.memset(h_cur[:], 0.0)
                c_cur = p1c.tile([128, 3, 32], F32, tag="c")
                nc.vector.memset(c_cur[:], 0.0)
                for m in range(3):
                    nc.sync.dma_start(cs[m].ap()[0], zero_t[:])

                h_fin = None
                for w in range(Tp // CH2):
                    t0 = w * CH2
                    win = p1w.tile([128, 12, CH2, 32], F32R, tag="xwbwin")
                    for s in range(12):
                        nc.sync.dma_start(
                            win[:, s, :, :],
                            xwb.ap()[s, t0:t0 + CH2, :, :].transpose([1, 0, 2]).bitcast(F32R))
                    for j in range(CH2):
                        t = t0 + j
                        gp = p1p.tile([128, 12, 32], F32, tag="gates")
                        nc.tensor.matmul(gp[:], id_t[:], win[:, :, j, :],
                                         start=True, stop=False)
                        for s in range(12):
                            gq, m = divmod(s, 3)
                            nc.tensor.matmul(
                                gp[:, s, :],
                                whhT_t[:, s * 128:(s + 1) * 128],
                                h_cur[:, m * 32:(m + 1) * 32],
                                start=False, stop=(s == 11))
                        sg = p1s.tile([128, 9, 32], F32, tag="sg")
                        nc.scalar.activation(sg[:], gp[:, 0:9, :], AF.Sigmoid)
                        tg = p1s.tile([128, 3, 32], F32, tag="tg")
                        nc.scalar.activation(tg[:], gp[:, 9:12, :], AF.Tanh)
                        t1 = p1s.tile([128, 3, 32], F32, tag="t1")
                        nc.vector.tensor_mul(t1[:], sg[:, 0:3, :], tg[:])
                        t2 = p1s.tile([128, 3, 32], F32, tag="t2")
                        nc.vector.tensor_mul(t2[:], sg[:, 3:6, :], c_cur[:])
                        c_new = p1c.tile([128, 3, 32], F32, tag="c")
                        nc.vector.tensor_add(c_new[:], t1[:], t2[:])
                        tc_t = p1s.tile([128, 3, 32], F32, tag="tc")
                        nc.scalar.activation(tc_t[:], c_new[:], AF.Tanh)
                        h_new = hpool.tile([128, 96], F32, tag="h")
                        nc.vector.tensor_mul(
                            h_new[:].rearrange("p (m b) -> p m b", b=32),
                            sg[:, 6:9, :], tc_t[:])
                        for m in range(3):
                            nc.sync.dma_start(cs[m].ap()[t + 1], c_new[:, m, :])
                        c_cur = c_new
                        h_cur = h_new
                h_fin = h_cur

            tc.strict_bb_all_engine_barrier()

            # =================== PHASE 2a: time-parallel attention ===================
            with (
                tc.tile_pool(name="p2c", bufs=2) as p2c,
                tc.tile_pool(name="p2s", bufs=2) as p2s,
                tc.tile_pool(name="p2r", bufs=3) as p2r,
                tc.tile_pool(name="p2p1", bufs=2, space="PSUM") as p2p1,
                tc.tile_pool(name="p2pe", bufs=1, space="PSUM") as p2pe,
                tc.tile_pool(name="p2po", bufs=2, space="PSUM") as p2po,
                tc.tile_pool(name="p2ps", bufs=1, space="PSUM") as p2ps,
            ):
                for k in range(NCH2):
                    t0 = k * CH2
                    cw = [p2c.tile([128, CH2 + 1, 32], F32R, tag=f"cw{m}", name=f"cw{m}")
                          for m in range(3)]
                    for m in range(3):
                        nc.sync.dma_start(
                            cw[m][:], cs[m].ap()[t0:t0 + CH2 + 1].transpose([1, 0, 2]).bitcast(F32R))

                    def rhs_k(kc):
                        if kc < 3:
                            return cw[kc][:, 0:CH2, :]
                        return cw[kc - 3][:, 1:CH2 + 1, :]

                    # att1 layer 1 + relu
                    y1p = p2p1.tile([128, 2, NB2], F32, tag="stage1")
                    for mc in range(2):
                        for kc in range(6):
                            nc.tensor.matmul(
                                y1p[:, mc, :], a1w1_t[:, kc, mc * 128:(mc + 1) * 128],
                                rhs_k(kc), start=(kc == 0), stop=(kc == 5))
                    y1 = p2s.tile([128, 2, NB2], F32R, tag="y1")
                    for mc in range(2):
                        nc.scalar.activation(y1[:, mc, :], y1p[:, mc, :], AF.Relu,
                                             bias=a1b1_t[:, mc:mc + 1])
                    # att1 layer 2 + exp
                    ep = p2pe.tile([128, 6, NB2], F32, tag="logits")
                    for mc6 in range(6):
                        for kc in range(2):
                            nc.tensor.matmul(
                                ep[:, mc6, :], a1w2_t[:, kc, mc6 * 128:(mc6 + 1) * 128],
                                y1[:, kc, :], start=(kc == 0), stop=(kc == 1))
                    et = p2s.tile([128, 6, NB2], F32R, tag="et")
                    for mc6 in range(6):
                        nc.scalar.activation(et[:, mc6, :], ep[:, mc6, :], AF.Exp,
                                             bias=a1b2_t[:, mc6:mc6 + 1])
                    # U = E * cStar (unnormalized attended)
                    ut = p2s.tile([128, 6, NB2], F32R, tag="ut")
                    for q in range(6):
                        nc.vector.tensor_mul(ut[:, q, :], et[:, q, :].bitcast(F32),
                                             rhs_k(q).bitcast(F32))
                    # S = sum over features of E
                    sp = p2ps.tile([1, NB2], F32, tag="srow")
                    for q in range(6):
                        nc.tensor.matmul(sp[:], ones128_t[:], et[:, q, :],
                                         start=(q == 0), stop=(q == 5))
                    srow = p2r.tile([1, NB2], F32R, tag="srow_s")
                    nc.vector.tensor_copy(srow[:], sp[:])
                    nc.sync.dma_start(s_d.ap()[k:k + 1, :], srow[:].bitcast(F32))

                    # att2 layer 1 + relu
                    zp = p2p1.tile([128, 2, NB2], F32, tag="stage1")
                    for mc in range(2):
                        for kc in range(6):
                            nc.tensor.matmul(
                                zp[:, mc, :], a2w1_t[:, kc, mc * 128:(mc + 1) * 128],
                                ut[:, kc, :], start=(kc == 0), stop=False)
                        nc.tensor.matmul(zp[:, mc, :], a2b1r_t[:, mc * 128:(mc + 1) * 128],
                                         srow[:], start=False, stop=True)
                    z = p2s.tile([128, 2, NB2], F32R, tag="z")
                    for mc in range(2):
                        nc.scalar.activation(z[:, mc, :], zp[:, mc, :], AF.Relu)
                    # att2 layer 2 (raw) + b2*S fold
                    ap2 = p2po.tile([128, 2, NB2], F32, tag="out")
                    for mc in range(2):
                        for kc in range(2):
                            nc.tensor.matmul(
                                ap2[:, mc, :], a2w2_t[:, kc, mc * 128:(mc + 1) * 128],
                                z[:, kc, :], start=(kc == 0), stop=False)
                        nc.tensor.matmul(ap2[:, mc, :], a2b2r_t[:, mc * 128:(mc + 1) * 128],
                                         srow[:], start=False, stop=True)
                    a2s = p2s.tile([128, 2, NB2], F32, tag="a2s")
                    nc.scalar.copy(a2s[:], ap2[:])
                    nc.sync.dma_start(a2r_d.ap()[k], a2s[:].rearrange("p a b -> p (a b)"))

                    # g1 / g2 attended-part + b1*S fold
                    for gi, (gw, gbr, gd) in enumerate(
                            ((g1a_t, g1b1r_t, g1p_d), (g2a_t, g2b1r_t, g2p_d))):
                        gp2 = p2po.tile([128, 2, NB2], F32, tag="out")
                        for mc in range(2):
                            for kc in range(6):
                                nc.tensor.matmul(
                                    gp2[:, mc, :], gw[:, kc, mc * 128:(mc + 1) * 128],
                                    ut[:, kc, :], start=(kc == 0), stop=False)
                            nc.tensor.matmul(gp2[:, mc, :], gbr[:, mc * 128:(mc + 1) * 128],
                                             srow[:], start=False, stop=True)
                        gs = p2s.tile([128, 2, NB2], F32, tag=f"g{gi}s")
                        if gi == 0:
                            nc.vector.tensor_copy(gs[:], gp2[:])
                        else:
                            nc.scalar.copy(gs[:], gp2[:])
                        nc.sync.dma_start(gd.ap()[k], gs[:].rearrange("p a b -> p (a b)"))

            tc.strict_bb_all_engine_barrier()

            # =================== reciprocal of S ===================
            with tc.tile_pool(name="prc", bufs=1) as prc:
                nrows = NCH2
                sall = prc.tile([nrows, NB2], F32)
                nc.sync.dma_start(sall[:], s_d.ap())
                sinv = prc.tile([nrows, NB2], F32)
                nc.vector.reciprocal(sinv[:], sall[:])
                nc.sync.dma_start(sinv_d.ap(), sinv[:])

            tc.strict_bb_all_engine_barrier()

            # =================== PHASE 3: memory recurrence ===================
            with (
                tc.tile_pool(name="p3w", bufs=2) as p3w,
                tc.tile_pool(name="p3s", bufs=3) as p3s,
                tc.tile_pool(name="p3m", bufs=2) as p3m,
                tc.tile_pool(name="p3p", bufs=2, space="PSUM") as p3p,
                tc.tile_pool(name="p3pb", bufs=2, space="PSUM") as p3pb,
            ):
                mem_cur = p3m.tile([128, 2, 32], F32, tag="mem")
                nc.vector.memset(mem_cur[:], 0.0)
                for k in range(NCH2):
                    aw = p3w.tile([128, 2, CH2, 32], F32, tag="aw")
                    nc.sync.dma_start(aw[:], a2r_d.ap()[k].rearrange("p (a t b) -> p a t b", a=2, b=32))
                    g1w_ = p3w.tile([128, 2, CH2, 32], F32, tag="g1w")
                    nc.sync.dma_start(g1w_[:], g1p_d.ap()[k].rearrange("p (a t b) -> p a t b", a=2, b=32))
                    g2w_ = p3w.tile([128, 2, CH2, 32], F32, tag="g2w")
                    nc.sync.dma_start(g2w_[:], g2p_d.ap()[k].rearrange("p (a t b) -> p a t b", a=2, b=32))
                    sr = p3w.tile([1, NB2], F32, tag="sr")
                    nc.sync.dma_start(sr[:], sinv_d.ap()[k:k + 1, :])

                    for j in range(CH2):
                        # broadcast Sinv_t across partitions via K=1 matmul
                        sb = p3pb.tile([128, 32], F32, tag="sinvb")
                        nc.tensor.matmul(sb[:], ones1x128_t[:], sr[:, j * 32:(j + 1) * 32],
                                         start=True, stop=True)
                        sb2 = sb[:].unsqueeze(1).broadcast_to([128, 2, 32])
                        # normalized g-pre parts
                        u = p3s.tile([128, 4, 32], F32, tag="u")
                        nc.vector.tensor_mul(u[:, 0:2, :], g1w_[:, :, j, :], sb2)
                        nc.vector.tensor_mul(u[:, 2:4, :], g2w_[:, :, j, :], sb2)
                        # mem-part matmuls (+ nothing else: b1*S already folded)
                        pg = p3p.tile([128, 4, 32], F32, tag="gmm")
                        for r, (gwt,) in enumerate(((g1b_t,), (g1b_t,), (g2b_t,), (g2b_t,))):
                            mc = r % 2
                            for kc in range(2):
                                nc.tensor.matmul(
                                    pg[:, r, :], gwt[:, kc, mc * 128:(mc + 1) * 128],
                                    mem_cur[:, kc, :], start=(kc == 0), stop=(kc == 1))
                        w_t = p3s.tile([128, 4, 32], F32, tag="w")
                        nc.vector.tensor_add(w_t[:], u[:], pg[:])
                        hh = p3s.tile([128, 4, 32], F32, tag="hh")
                        nc.scalar.activation(hh[:], w_t[:], AF.Relu)
                        # L2 + b2 fold
                        qg = p3p.tile([128, 4, 32], F32, tag="qmm")
                        for r, gwt in enumerate((g1w2_t, g1w2_t, g2w2_t, g2w2_t)):
                            mc = r % 2
                            goff = 0 if r < 2 else 2
                            for kc in range(2):
                                nc.tensor.matmul(
                                    qg[:, r, :], gwt[:, kc, mc * 128:(mc + 1) * 128],
                                    hh[:, goff + kc, :], start=(kc == 0), stop=False)
                            nc.tensor.matmul(qg[:, r, :], gb2r_t[:, r * 128:(r + 1) * 128],
                                             ones32_t[:], start=False, stop=True)
                        gam = p3s.tile([128, 4, 32], F32, tag="gam")
                        nc.scalar.activation(gam[:], qg[:], AF.Sigmoid)
                        # cHat = tanh(att2raw * Sinv)  (b2*S folded already)
                        v = p3s.tile([128, 2, 32], F32, tag="v")
                        nc.vector.tensor_mul(v[:], aw[:, :, j, :], sb2)
                        ch = p3s.tile([128, 2, 32], F32, tag="ch")
                        nc.scalar.activation(ch[:], v[:], AF.Tanh)
                        # mem = gam1*mem + gam2*cHat
                        m1 = p3s.tile([128, 2, 32], F32, tag="m1")
                        nc.vector.tensor_mul(m1[:], gam[:, 0:2, :], mem_cur[:])
                        m2 = p3s.tile([128, 2, 32], F32, tag="m2")
                        nc.vector.tensor_mul(m2[:], gam[:, 2:4, :], ch[:])
                        mem_new = p3m.tile([128, 2, 32], F32, tag="mem")
                        nc.vector.tensor_add(mem_new[:], m1[:], m2[:])
                        mem_cur = mem_new

                # =================== PHASE 4: output MLP ===================
                with tc.tile_pool(name="p4p", bufs=1, space="PSUM") as p4p:
                    o1p = p4p.tile([128, 2, 32], F32, tag="o1")
                    rhs5 = [h_fin[:, 0:32], h_fin[:, 32:64], h_fin[:, 64:96],
                            mem_cur[:, 0, :], mem_cur[:, 1, :]]
                    for mc in range(2):
                        for kc in range(5):
                            nc.tensor.matmul(
                                o1p[:, mc, :], ow1_t[:, kc, mc * 128:(mc + 1) * 128],
                                rhs5[kc], start=(kc == 0), stop=(kc == 4))
                    o1s = p3s.tile([128, 2, 32], F32, tag="o1s")
                    for mc in range(2):
                        nc.scalar.activation(o1s[:, mc, :], o1p[:, mc, :], AF.Relu,
                                             bias=ob1_t[:, mc:mc + 1])
                    o2p = p4p.tile([1, 32], F32, tag="o2")
                    for kc in range(2):
                        nc.tensor.matmul(o2p[:], ow2_t[:, kc, :], o1s[:, kc, :],
                                         start=(kc == 0), stop=(kc == 1))
                    o2s = p3s.tile([1, 32], F32, tag="o2s")
                    nc.scalar.activation(o2s[:], o2p[:], AF.Identity, bias=ob2_t[:])
                    nc.sync.dma_start(out_d.ap().rearrange("b one -> (one) (b)"), o2s[:])

    nc.compile()
    return nc


# ---------------------------------------------------------------------------
# host-side packing
# ---------------------------------------------------------------------------

def pack_shared(inp):
    """Pack weight tensors (identical across cores)."""
    f = np.float32
    d = {}
    wih = {0: inp["Wih_l"], 1: inp["Wih_a"], 2: inp["Wih_v"]}
    whh = {0: inp["Whh_l"], 1: inp["Whh_a"], 2: inp["Whh_v"]}
    bb = {m: (inp[f"bih_{k}"] + inp[f"bhh_{k}"]).astype(f)
          for m, k in ((0, "l"), (1, "a"), (2, "v"))}
    foff = {0: 0, 1: D_L, 2: D_L + D_A}
    din = {0: D_L, 1: D_A, 2: D_V}

    waug = np.zeros((512, 1536), f)
    whhT = np.zeros((128, 1536), f)
    for gq in range(4):
        tg = TORCH_G[gq]
        for m in range(3):
            s = gq * 3 + m
            wblk = wih[m][tg * 128:(tg + 1) * 128, :]          # [128, din]
            waug[foff[m]:foff[m] + din[m], s * 128:(s + 1) * 128] = wblk.T
            waug[DIN, s * 128:(s + 1) * 128] = bb[m][tg * 128:(tg + 1) * 128]
            whhT[:, s * 128:(s + 1) * 128] = whh[m][tg * 128:(tg + 1) * 128, :].T
    d["waug"] = waug
    d["whhT"] = whhT
    d["ident"] = np.eye(128, dtype=f)
    d["ones128"] = np.ones((128, 1), f)

    d["a1w1"] = inp["att1_W1"].T.astype(f).copy()              # [768, 256]
    d["a1b1"] = inp["att1_b1"].reshape(2, 128).T.astype(f).copy()
    d["a1w2"] = inp["att1_W2"].T.astype(f).copy()              # [256, 768]
    d["a1b2"] = inp["att1_b2"].reshape(6, 128).T.astype(f).copy()
    d["a2w1"] = inp["att2_W1"].T.astype(f).copy()
    d["a2b1r"] = inp["att2_b1"].reshape(1, 256).astype(f).copy()
    d["a2w2"] = inp["att2_W2"].T.astype(f).copy()              # [256, 256]
    d["a2b2r"] = inp["att2_b2"].reshape(1, 256).astype(f).copy()
    d["g1a"] = inp["g1_W1"][:, :768].T.astype(f).copy()
    d["g2a"] = inp["g2_W1"][:, :768].T.astype(f).copy()
    d["g1b"] = inp["g1_W1"][:, 768:].T.astype(f).copy()
    d["g2b"] = inp["g2_W1"][:, 768:].T.astype(f).copy()
    d["g1b1r"] = inp["g1_b1"].reshape(1, 256).astype(f).copy()
    d["g2b1r"] = inp["g2_b1"].reshape(1, 256).astype(f).copy()
    d["g1w2"] = inp["g1_W2"].T.astype(f).copy()
    d["g2w2"] = inp["g2_W2"].T.astype(f).copy()
    d["gb2r"] = np.concatenate([inp["g1_b2"], inp["g2_b2"]]).reshape(1, 512).astype(f)
    d["ow1"] = inp["out_W1"].T.astype(f).copy()                # [640, 256]
    d["ob1"] = inp["out_b1"].reshape(2, 128).T.astype(f).copy()
    d["ow2"] = inp["out_W2"].T.astype(f).copy()                # [256, 1]
    d["ob2"] = inp["out_b2"].reshape(1, 1).astype(f).copy()
    return d


def pack_x(x, core, Tp):
    """x: [Tp, 256, 409] -> xT [410, Tp*B] for one core."""
    xc = np.asarray(x[:, core * B:(core + 1) * B, :], np.float32)   # [Tp, B, 409]
    xt = xc.transpose(2, 0, 1).reshape(DIN, Tp * B)
    return np.concatenate([xt, np.ones((1, Tp * B), np.float32)], 0)


_CACHE = {}


def _get_program(Tp):
    if Tp not in _CACHE:
        _CACHE[Tp] = build_program(Tp)
    return _CACHE[Tp]


def kernel(**inputs):
    x = np.asarray(inputs["x"])
    Tp = x.shape[0]
    nc = _get_program(Tp)
    shared = pack_shared({k: np.asarray(v) for k, v in inputs.items()})
    in_maps = []
    for c in range(NCORES):
        m = dict(shared)
        m["xT"] = np.ascontiguousarray(pack_x(x, c, Tp))
        in_maps.append(m)
    res = run_bass_kernel_spmd(nc, in_maps, list(range(NCORES))).results
    out = np.concatenate([r["out"] for r in res], axis=0)
    return out.astype(np.float32)


if __name__ == "__main__":
    import time
    t0 = time.time()
    nc = build_program(32)
    print("built in", time.time() - t0, "s")



# revision 2
# speedup vs baseline: 2.3160x; 2.3160x over previous
"""Trainium2 Bass kernel for nn_Contextual_MFN (Memory Fusion Network).

Structure (per core; batch data-parallel 8 ways, 32 rows/core):
  phase 0: xWb[t] = Wih_aug @ x_aug[t]  (all t, bf16 matmuls, bias folded in)
  phase 1: sequential 3xLSTM recurrence; gates = xWb (identity-inject) + Whh@h
  phase 2a: time-parallel attention: att1 MLP -> exp -> U = E*cStar (unnormalized),
            S = sum(E), att2/g1/g2 linear parts on U, bias*S folds
  recip:   Sinv = 1/S
  phase 3: sequential memory-gate recurrence (mem-dependent matmuls only)
  phase 4: output MLP on [h_l, h_a, h_v, mem]

All activations feature-major: [features(partitions), batch(free)].
All matmuls bf16 (FWL weight loads, single-pass); psum accumulate fp32.
"""
import os
import numpy as np
import ml_dtypes

import concourse.bass as bass
import concourse.tile as tile
from concourse import bacc, mybir
from concourse.bass_utils import run_bass_kernel_spmd

F32 = mybir.dt.float32
BF = mybir.dt.bfloat16
AF = mybir.ActivationFunctionType
NPBF = ml_dtypes.bfloat16

# Problem constants (hardcoded; kernel.py must be self-contained)
T_FULL = 512
NBATCH = 256
NCORES = 8
B = NBATCH // NCORES          # 32 batch rows per core
D_L, D_A, D_V = 300, 74, 35
DIN = D_L + D_A + D_V         # 409
DAUG = DIN + 1                # 410 (ones row for bias)
DH = 128
MEM = 256
CH0 = 16                      # phase-0 chunk (steps)
CH2 = 8                       # phase-2a / phase-3 chunk (steps)

# gate slot order: s = g'*3 + m, with g' in (i, f, o, g_tanh); torch rows are (i, f, g, o)
TORCH_G = (0, 1, 3, 2)        # our slot g' -> torch gate row block


def _nonzero_kcs(s):
    """Phase-0 K-chunks (of Waug rows 0..409 padded to 512) that are nonzero for
    output slot s. m=0 (l): feats 0-299 -> kc 0,1,2 (+ones kc3). m=1 (a): 300-373
    -> kc2 (+kc3 ones). m=2 (v): 374-408 -> kc2,kc3 (+ones kc3)."""
    m = s % 3
    if m == 0:
        return [0, 1, 2, 3]
    return [2, 3]


def build_program(Tp=T_FULL):
    assert Tp % CH0 == 0 and Tp % CH2 == 0
    NCH2 = Tp // CH2
    nc = bacc.Bacc("TRN2", target_bir_lowering=False, debug=False)

    # ---------------- external inputs ----------------
    xT = nc.dram_tensor("xT", [DAUG, Tp * B], BF, kind="ExternalInput")
    waug = nc.dram_tensor("waug", [512, 1536], BF, kind="ExternalInput")
    whhT = nc.dram_tensor("whhT", [128, 1536], BF, kind="ExternalInput")
    ident = nc.dram_tensor("ident", [128, 128], BF, kind="ExternalInput")
    ones128 = nc.dram_tensor("ones128", [128, 1], BF, kind="ExternalInput")

    a1w1 = nc.dram_tensor("a1w1", [768, 256], BF, kind="ExternalInput")
    a1b1 = nc.dram_tensor("a1b1", [128, 2], F32, kind="ExternalInput")
    a1w2 = nc.dram_tensor("a1w2", [256, 768], BF, kind="ExternalInput")
    a1b2 = nc.dram_tensor("a1b2", [128, 6], F32, kind="ExternalInput")
    a2w1 = nc.dram_tensor("a2w1", [768, 256], BF, kind="ExternalInput")
    a2b1r = nc.dram_tensor("a2b1r", [1, 256], BF, kind="ExternalInput")
    a2w2 = nc.dram_tensor("a2w2", [256, 256], BF, kind="ExternalInput")
    a2b2r = nc.dram_tensor("a2b2r", [1, 256], BF, kind="ExternalInput")
    g1a = nc.dram_tensor("g1a", [768, 256], BF, kind="ExternalInput")
    g2a = nc.dram_tensor("g2a", [768, 256], BF, kind="ExternalInput")
    g1b = nc.dram_tensor("g1b", [256, 256], BF, kind="ExternalInput")
    g2b = nc.dram_tensor("g2b", [256, 256], BF, kind="ExternalInput")
    g1b1r = nc.dram_tensor("g1b1r", [1, 256], BF, kind="ExternalInput")
    g2b1r = nc.dram_tensor("g2b1r", [1, 256], BF, kind="ExternalInput")
    g1w2 = nc.dram_tensor("g1w2", [256, 256], BF, kind="ExternalInput")
    g2w2 = nc.dram_tensor("g2w2", [256, 256], BF, kind="ExternalInput")
    gb2c = nc.dram_tensor("gb2c", [128, 4], F32, kind="ExternalInput")
    ow1 = nc.dram_tensor("ow1", [640, 256], BF, kind="ExternalInput")
    ob1 = nc.dram_tensor("ob1", [128, 2], F32, kind="ExternalInput")
    ow2 = nc.dram_tensor("ow2", [256, 1], BF, kind="ExternalInput")
    ob2 = nc.dram_tensor("ob2", [1, 1], F32, kind="ExternalInput")

    out_d = nc.dram_tensor("out", [B, 1], F32, kind="ExternalOutput")

    # ---------------- internal dram scratch ----------------
    xwb = nc.dram_tensor("xwb", [12, Tp, 128, B], BF)
    cs = [nc.dram_tensor(f"cseq{m}", [Tp + 1, 128, B], BF) for m in range(3)]
    a2r_d = nc.dram_tensor("a2r_d", [NCH2, 128, 2 * CH2 * B], BF)
    g1p_d = nc.dram_tensor("g1p_d", [NCH2, 128, 2 * CH2 * B], BF)
    g2p_d = nc.dram_tensor("g2p_d", [NCH2, 128, 2 * CH2 * B], BF)
    s_d = nc.dram_tensor("s_d", [NCH2, CH2 * B], F32)
    sinv_d = nc.dram_tensor("sinv_d", [NCH2, CH2 * B], BF)

    NB2 = CH2 * B  # 256: phase-2a matmul free dim

    import contextlib
    with tile.TileContext(nc) as tc:
        ctx = contextlib.ExitStack()
        with ctx:
            wpool = ctx.enter_context(tc.tile_pool(name="weights", bufs=1))
            hpool = ctx.enter_context(tc.tile_pool(name="hstate", bufs=2))

            # ---- resident weights / constants in SBUF ----
            wihT_t = wpool.tile([128, 4, 1536], BF)
            nc.sync.dma_start(
                wihT_t[:], waug.ap().rearrange("(kc p) c -> p kc c", p=128))
            whhT_t = wpool.tile([128, 1536], BF)
            nc.sync.dma_start(whhT_t[:], whhT.ap())
            id_t = wpool.tile([128, 128], BF)
            nc.sync.dma_start(id_t[:], ident.ap())
            ones128_t = wpool.tile([128, 1], BF)
            nc.sync.dma_start(ones128_t[:], ones128.ap())

            a1w1_t = wpool.tile([128, 6, 256], BF)
            nc.sync.dma_start(a1w1_t[:], a1w1.ap().rearrange("(kc p) c -> p kc c", p=128))
            a1b1_t = wpool.tile([128, 2], F32)
            nc.sync.dma_start(a1b1_t[:], a1b1.ap())
            a1w2_t = wpool.tile([128, 2, 768], BF)
            nc.sync.dma_start(a1w2_t[:], a1w2.ap().rearrange("(kc p) c -> p kc c", p=128))
            a1b2_t = wpool.tile([128, 6], F32)
            nc.sync.dma_start(a1b2_t[:], a1b2.ap())
            a2w1_t = wpool.tile([128, 6, 256], BF)
            nc.sync.dma_start(a2w1_t[:], a2w1.ap().rearrange("(kc p) c -> p kc c", p=128))
            a2b1r_t = wpool.tile([1, 256], BF)
            nc.sync.dma_start(a2b1r_t[:], a2b1r.ap())
            a2w2_t = wpool.tile([128, 2, 256], BF)
            nc.sync.dma_start(a2w2_t[:], a2w2.ap().rearrange("(kc p) c -> p kc c", p=128))
            a2b2r_t = wpool.tile([1, 256], BF)
            nc.sync.dma_start(a2b2r_t[:], a2b2r.ap())
            g1a_t = wpool.tile([128, 6, 256], BF)
            nc.sync.dma_start(g1a_t[:], g1a.ap().rearrange("(kc p) c -> p kc c", p=128))
            g2a_t = wpool.tile([128, 6, 256], BF)
            nc.sync.dma_start(g2a_t[:], g2a.ap().rearrange("(kc p) c -> p kc c", p=128))
            g1b1r_t = wpool.tile([1, 256], BF)
            nc.sync.dma_start(g1b1r_t[:], g1b1r.ap())
            g2b1r_t = wpool.tile([1, 256], BF)
            nc.sync.dma_start(g2b1r_t[:], g2b1r.ap())
            g1b_t = wpool.tile([128, 2, 256], BF)
            nc.sync.dma_start(g1b_t[:], g1b.ap().rearrange("(kc p) c -> p kc c", p=128))
            g2b_t = wpool.tile([128, 2, 256], BF)
            nc.sync.dma_start(g2b_t[:], g2b.ap().rearrange("(kc p) c -> p kc c", p=128))
            g1w2_t = wpool.tile([128, 2, 256], BF)
            nc.sync.dma_start(g1w2_t[:], g1w2.ap().rearrange("(kc p) c -> p kc c", p=128))
            g2w2_t = wpool.tile([128, 2, 256], BF)
            nc.sync.dma_start(g2w2_t[:], g2w2.ap().rearrange("(kc p) c -> p kc c", p=128))
            gb2_t = wpool.tile([128, 4], F32)
            nc.sync.dma_start(gb2_t[:], gb2c.ap())
            ow1_t = wpool.tile([128, 5, 256], BF)
            nc.sync.dma_start(ow1_t[:], ow1.ap().rearrange("(kc p) c -> p kc c", p=128))
            ob1_t = wpool.tile([128, 2], F32)
            nc.sync.dma_start(ob1_t[:], ob1.ap())
            ow2_t = wpool.tile([128, 2, 1], BF)
            nc.sync.dma_start(ow2_t[:], ow2.ap().rearrange("(kc p) c -> p kc c", p=128))
            ob2_t = wpool.tile([1, 1], F32)
            nc.sync.dma_start(ob2_t[:], ob2.ap())

            ones1x128_t = wpool.tile([1, 128], BF)
            nc.vector.memset(ones1x128_t[:], 1.0)
            zero_t = wpool.tile([128, 32], BF)
            nc.vector.memset(zero_t[:], 0.0)

            # =================== PHASE 0: xWb ===================
            with (
                tc.tile_pool(name="p0x", bufs=2) as p0x,
                tc.tile_pool(name="p0s", bufs=4) as p0s,
                tc.tile_pool(name="p0p", bufs=4, space="PSUM") as p0p,
            ):
                for k0 in range(Tp // CH0):
                    t0 = k0 * CH0
                    n0 = CH0 * B  # 512
                    xt = p0x.tile([128, 4, n0], BF, tag="xt")
                    for kc in range(4):
                        rows = 128 if kc < 3 else DAUG - 384  # 26 on last chunk
                        nc.sync.dma_start(
                            xt[0:rows, kc, :],
                            xT.ap()[kc * 128:kc * 128 + rows, t0 * B:(t0 + CH0) * B])
                    for s in range(12):
                        pt = p0p.tile([128, n0], F32, tag="p0acc")
                        kcs = _nonzero_kcs(s)
                        for i, kc in enumerate(kcs):
                            rows = 128 if kc < 3 else DAUG - 384
                            nc.tensor.matmul(
                                pt[:], wihT_t[0:rows, kc, s * 128:(s + 1) * 128],
                                xt[0:rows, kc, :],
                                start=(i == 0), stop=(i == len(kcs) - 1))
                        st = p0s.tile([128, n0], BF, tag="p0st")
                        if s % 2 == 0:
                            nc.vector.tensor_copy(st[:], pt[:])
                        else:
                            nc.scalar.copy(st[:], pt[:])
                        # dram [CH0, 128, B] slab, partition-major write
                        nc.sync.dma_start(
                            xwb.ap()[s, t0:t0 + CH0, :, :].transpose([1, 0, 2]),
                            st[:].rearrange("p (t b) -> p t b", b=B))

            tc.strict_bb_all_engine_barrier()

            # =================== PHASE 1: LSTM recurrence ===================
            with (
                tc.tile_pool(name="p1w", bufs=2) as p1w,
                tc.tile_pool(name="p1s", bufs=3) as p1s,
                tc.tile_pool(name="p1c", bufs=4) as p1c,
                tc.tile_pool(name="p1p", bufs=2, space="PSUM") as p1p,
            ):
                h_cur = hpool.tile([128, 96], BF, tag="h")
                nc.vector.memset(h_cur[:], 0.0)
                c_cur = p1c.tile([128, 3, 32], BF, tag="c")
                nc.vector.memset(c_cur[:], 0.0)
                for m in range(3):
                    nc.sync.dma_start(cs[m].ap()[0], zero_t[:])

                h_fin = None
                for w in range(Tp // CH2):
                    t0 = w * CH2
                    win = p1w.tile([128, 12, CH2, 32], BF, tag="xwbwin")
                    for s in range(12):
                        nc.sync.dma_start(
                            win[:, s, :, :],
                            xwb.ap()[s, t0:t0 + CH2, :, :].transpose([1, 0, 2]))
                    for j in range(CH2):
                        t = t0 + j
                        gp = p1p.tile([128, 12, 32], F32, tag="gates")
                        nc.tensor.matmul(gp[:], id_t[:], win[:, :, j, :],
                                         start=True, stop=False)
                        for s in range(12):
                            gq, m = divmod(s, 3)
                            nc.tensor.matmul(
                                gp[:, s, :],
                                whhT_t[:, s * 128:(s + 1) * 128],
                                h_cur[:, m * 32:(m + 1) * 32],
                                start=False, stop=(s == 11))
                        sg = p1s.tile([128, 9, 32], BF, tag="sg")
                        nc.scalar.activation(sg[:], gp[:, 0:9, :], AF.Sigmoid)
                        tg = p1s.tile([128, 3, 32], BF, tag="tg")
                        nc.scalar.activation(tg[:], gp[:, 9:12, :], AF.Tanh)
                        t1 = p1s.tile([128, 3, 32], BF, tag="t1")
                        nc.vector.tensor_mul(t1[:], sg[:, 0:3, :], tg[:])
                        t2 = p1s.tile([128, 3, 32], BF, tag="t2")
                        nc.vector.tensor_mul(t2[:], sg[:, 3:6, :], c_cur[:])
                        c_new = p1c.tile([128, 3, 32], BF, tag="c")
                        nc.vector.tensor_add(c_new[:], t1[:], t2[:])
                        tc_t = p1s.tile([128, 3, 32], BF, tag="tc")
                        nc.scalar.activation(tc_t[:], c_new[:], AF.Tanh)
                        h_new = hpool.tile([128, 96], BF, tag="h")
                        nc.vector.tensor_mul(
                            h_new[:].rearrange("p (m b) -> p m b", b=32),
                            sg[:, 6:9, :], tc_t[:])
                        for m in range(3):
                            nc.sync.dma_start(cs[m].ap()[t + 1], c_new[:, m, :])
                        c_cur = c_new
                        h_cur = h_new
                h_fin = h_cur

            tc.strict_bb_all_engine_barrier()

            # =================== PHASE 2a: time-parallel attention ===================
            with (
                tc.tile_pool(name="p2c", bufs=2) as p2c,
                tc.tile_pool(name="p2s", bufs=2) as p2s,
                tc.tile_pool(name="p2r", bufs=3) as p2r,
                tc.tile_pool(name="p2p1", bufs=2, space="PSUM") as p2p1,
                tc.tile_pool(name="p2pe", bufs=1, space="PSUM") as p2pe,
                tc.tile_pool(name="p2po", bufs=2, space="PSUM") as p2po,
                tc.tile_pool(name="p2ps", bufs=1, space="PSUM") as p2ps,
            ):
                for k in range(NCH2):
                    t0 = k * CH2
                    cw = [p2c.tile([128, CH2 + 1, 32], BF, tag=f"cw{m}", name=f"cw{m}")
                          for m in range(3)]
                    for m in range(3):
                        nc.sync.dma_start(
                            cw[m][:], cs[m].ap()[t0:t0 + CH2 + 1].transpose([1, 0, 2]))

                    def rhs_k(kc):
                        if kc < 3:
                            return cw[kc][:, 0:CH2, :]
                        return cw[kc - 3][:, 1:CH2 + 1, :]

                    # att1 layer 1 + relu
                    y1p = p2p1.tile([128, 2, NB2], F32, tag="stage1")
                    for mc in range(2):
                        for kc in range(6):
                            nc.tensor.matmul(
                                y1p[:, mc, :], a1w1_t[:, kc, mc * 128:(mc + 1) * 128],
                                rhs_k(kc), start=(kc == 0), stop=(kc == 5))
                    y1 = p2s.tile([128, 2, NB2], BF, tag="y1")
                    for mc in range(2):
                        nc.scalar.activation(y1[:, mc, :], y1p[:, mc, :], AF.Relu,
                                             bias=a1b1_t[:, mc:mc + 1])
                    # att1 layer 2 + exp
                    ep = p2pe.tile([128, 6, NB2], F32, tag="logits")
                    for mc6 in range(6):
                        for kc in range(2):
                            nc.tensor.matmul(
                                ep[:, mc6, :], a1w2_t[:, kc, mc6 * 128:(mc6 + 1) * 128],
                                y1[:, kc, :], start=(kc == 0), stop=(kc == 1))
                    et = p2s.tile([128, 6, NB2], BF, tag="et")
                    for mc6 in range(6):
                        nc.scalar.activation(et[:, mc6, :], ep[:, mc6, :], AF.Exp,
                                             bias=a1b2_t[:, mc6:mc6 + 1])
                    # U = E * cStar (unnormalized attended)
                    ut = p2s.tile([128, 6, NB2], BF, tag="ut")
                    for q in range(6):
                        nc.vector.tensor_mul(ut[:, q, :], et[:, q, :], rhs_k(q))
                    # S = sum over features of E
                    sp = p2ps.tile([1, NB2], F32, tag="srow")
                    for q in range(6):
                        nc.tensor.matmul(sp[:], ones128_t[:], et[:, q, :],
                                         start=(q == 0), stop=(q == 5))
                    srow = p2r.tile([1, NB2], BF, tag="srow_s")
                    nc.vector.tensor_copy(srow[:], sp[:])
                    sf32 = p2r.tile([1, NB2], F32, tag="srow_f")
                    nc.scalar.copy(sf32[:], sp[:])
                    nc.sync.dma_start(s_d.ap()[k:k + 1, :], sf32[:])

                    # att2 layer 1 + relu
                    zp = p2p1.tile([128, 2, NB2], F32, tag="stage1")
                    for mc in range(2):
                        for kc in range(6):
                            nc.tensor.matmul(
                                zp[:, mc, :], a2w1_t[:, kc, mc * 128:(mc + 1) * 128],
                                ut[:, kc, :], start=(kc == 0), stop=False)
                        nc.tensor.matmul(zp[:, mc, :], a2b1r_t[:, mc * 128:(mc + 1) * 128],
                                         srow[:], start=False, stop=True)
                    z = p2s.tile([128, 2, NB2], BF, tag="z")
                    for mc in range(2):
                        nc.scalar.activation(z[:, mc, :], zp[:, mc, :], AF.Relu)
                    # att2 layer 2 (raw) + b2*S fold
                    ap2 = p2po.tile([128, 2, NB2], F32, tag="out")
                    for mc in range(2):
                        for kc in range(2):
                            nc.tensor.matmul(
                                ap2[:, mc, :], a2w2_t[:, kc, mc * 128:(mc + 1) * 128],
                                z[:, kc, :], start=(kc == 0), stop=False)
                        nc.tensor.matmul(ap2[:, mc, :], a2b2r_t[:, mc * 128:(mc + 1) * 128],
                                         srow[:], start=False, stop=True)
                    a2s = p2s.tile([128, 2, NB2], BF, tag="a2s")
                    nc.scalar.copy(a2s[:], ap2[:])
                    nc.sync.dma_start(a2r_d.ap()[k], a2s[:].rearrange("p a b -> p (a b)"))

                    # g1 / g2 attended-part + b1*S fold
                    for gi, (gw, gbr, gd) in enumerate(
                            ((g1a_t, g1b1r_t, g1p_d), (g2a_t, g2b1r_t, g2p_d))):
                        gp2 = p2po.tile([128, 2, NB2], F32, tag="out")
                        for mc in range(2):
                            for kc in range(6):
                                nc.tensor.matmul(
                                    gp2[:, mc, :], gw[:, kc, mc * 128:(mc + 1) * 128],
                                    ut[:, kc, :], start=(kc == 0), stop=False)
                            nc.tensor.matmul(gp2[:, mc, :], gbr[:, mc * 128:(mc + 1) * 128],
                                             srow[:], start=False, stop=True)
                        gs = p2s.tile([128, 2, NB2], BF, tag=f"g{gi}s")
                        if gi == 0:
                            nc.vector.tensor_copy(gs[:], gp2[:])
                        else:
                            nc.scalar.copy(gs[:], gp2[:])
                        nc.sync.dma_start(gd.ap()[k], gs[:].rearrange("p a b -> p (a b)"))

            tc.strict_bb_all_engine_barrier()

            # =================== reciprocal of S ===================
            with tc.tile_pool(name="prc", bufs=1) as prc:
                nrows = NCH2
                sall = prc.tile([nrows, NB2], F32)
                nc.sync.dma_start(sall[:], s_d.ap())
                sinv = prc.tile([nrows, NB2], F32)
                nc.vector.reciprocal(sinv[:], sall[:])
                sinvb = prc.tile([nrows, NB2], BF)
                nc.vector.tensor_copy(sinvb[:], sinv[:])
                nc.sync.dma_start(sinv_d.ap(), sinvb[:])

            tc.strict_bb_all_engine_barrier()

            # =================== PHASE 3: memory recurrence ===================
            with (
                tc.tile_pool(name="p3w", bufs=2) as p3w,
                tc.tile_pool(name="p3s", bufs=3) as p3s,
                tc.tile_pool(name="p3m", bufs=2) as p3m,
                tc.tile_pool(name="p3p", bufs=2, space="PSUM") as p3p,
                tc.tile_pool(name="p3pb", bufs=2, space="PSUM") as p3pb,
            ):
                mem_cur = p3m.tile([128, 2, 32], BF, tag="mem")
                nc.vector.memset(mem_cur[:], 0.0)
                for k in range(NCH2):
                    aw = p3w.tile([128, 2, CH2, 32], BF, tag="aw")
                    nc.sync.dma_start(aw[:], a2r_d.ap()[k].rearrange("p (a t b) -> p a t b", a=2, b=32))
                    g1w_ = p3w.tile([128, 2, CH2, 32], BF, tag="g1w")
                    nc.sync.dma_start(g1w_[:], g1p_d.ap()[k].rearrange("p (a t b) -> p a t b", a=2, b=32))
                    g2w_ = p3w.tile([128, 2, CH2, 32], BF, tag="g2w")
                    nc.sync.dma_start(g2w_[:], g2p_d.ap()[k].rearrange("p (a t b) -> p a t b", a=2, b=32))
                    sr = p3w.tile([1, NB2], BF, tag="sr")
                    nc.sync.dma_start(sr[:], sinv_d.ap()[k:k + 1, :])

                    # broadcast Sinv across partitions for the whole window (1 matmul)
                    sbp = p3pb.tile([128, NB2], F32, tag="sinvb")
                    nc.tensor.matmul(sbp[:], ones1x128_t[:], sr[:],
                                     start=True, stop=True)
                    sb_w = p3w.tile([128, NB2], BF, tag="sbw")
                    nc.scalar.copy(sb_w[:], sbp[:])

                    for j in range(CH2):
                        sb2 = sb_w[:, j * 32:(j + 1) * 32].unsqueeze(1).broadcast_to([128, 2, 32])
                        # normalized g-pre parts
                        u = p3s.tile([128, 4, 32], BF, tag="u")
                        nc.vector.tensor_mul(u[:, 0:2, :], g1w_[:, :, j, :], sb2)
                        nc.vector.tensor_mul(u[:, 2:4, :], g2w_[:, :, j, :], sb2)
                        # mem-part matmuls (+ nothing else: b1*S already folded)
                        pg = p3p.tile([128, 4, 32], F32, tag="gmm")
                        for r, (gwt,) in enumerate(((g1b_t,), (g1b_t,), (g2b_t,), (g2b_t,))):
                            mc = r % 2
                            for kc in range(2):
                                nc.tensor.matmul(
                                    pg[:, r, :], gwt[:, kc, mc * 128:(mc + 1) * 128],
                                    mem_cur[:, kc, :], start=(kc == 0), stop=(kc == 1))
                        w_t = p3s.tile([128, 4, 32], BF, tag="w")
                        nc.vector.tensor_add(w_t[:], u[:], pg[:])
                        hh = p3s.tile([128, 4, 32], BF, tag="hh")
                        nc.scalar.activation(hh[:], w_t[:], AF.Relu)
                        # L2; b2 folded into sigmoid bias
                        qg = p3p.tile([128, 4, 32], F32, tag="qmm")
                        for r, gwt in enumerate((g1w2_t, g1w2_t, g2w2_t, g2w2_t)):
                            mc = r % 2
                            goff = 0 if r < 2 else 2
                            for kc in range(2):
                                nc.tensor.matmul(
                                    qg[:, r, :], gwt[:, kc, mc * 128:(mc + 1) * 128],
                                    hh[:, goff + kc, :], start=(kc == 0), stop=(kc == 1))
                        gam = p3s.tile([128, 4, 32], BF, tag="gam")
                        for r in range(4):
                            nc.scalar.activation(gam[:, r, :], qg[:, r, :], AF.Sigmoid,
                                                 bias=gb2_t[:, r:r + 1])
                        # cHat = tanh(att2raw * Sinv)  (b2*S folded already)
                        v = p3s.tile([128, 2, 32], BF, tag="v")
                        nc.vector.tensor_mul(v[:], aw[:, :, j, :], sb2)
                        ch = p3s.tile([128, 2, 32], BF, tag="ch")
                        nc.scalar.activation(ch[:], v[:], AF.Tanh)
                        # mem = gam1*mem + gam2*cHat
                        m1 = p3s.tile([128, 2, 32], BF, tag="m1")
                        nc.vector.tensor_mul(m1[:], gam[:, 0:2, :], mem_cur[:])
                        m2 = p3s.tile([128, 2, 32], BF, tag="m2")
                        nc.vector.tensor_mul(m2[:], gam[:, 2:4, :], ch[:])
                        mem_new = p3m.tile([128, 2, 32], BF, tag="mem")
                        nc.vector.tensor_add(mem_new[:], m1[:], m2[:])
                        mem_cur = mem_new

                # =================== PHASE 4: output MLP ===================
                with tc.tile_pool(name="p4p", bufs=1, space="PSUM") as p4p:
                    o1p = p4p.tile([128, 2, 32], F32, tag="o1")
                    rhs5 = [h_fin[:, 0:32], h_fin[:, 32:64], h_fin[:, 64:96],
                            mem_cur[:, 0, :], mem_cur[:, 1, :]]
                    for mc in range(2):
                        for kc in range(5):
                            nc.tensor.matmul(
                                o1p[:, mc, :], ow1_t[:, kc, mc * 128:(mc + 1) * 128],
                                rhs5[kc], start=(kc == 0), stop=(kc == 4))
                    o1s = p3s.tile([128, 2, 32], BF, tag="o1s")
                    for mc in range(2):
                        nc.scalar.activation(o1s[:, mc, :], o1p[:, mc, :], AF.Relu,
                                             bias=ob1_t[:, mc:mc + 1])
                    o2p = p4p.tile([1, 32], F32, tag="o2")
                    for kc in range(2):
                        nc.tensor.matmul(o2p[:], ow2_t[:, kc, :], o1s[:, kc, :],
                                         start=(kc == 0), stop=(kc == 1))
                    o2s = p3s.tile([1, 32], F32, tag="o2s")
                    nc.scalar.activation(o2s[:], o2p[:], AF.Identity, bias=ob2_t[:])
                    nc.sync.dma_start(out_d.ap().rearrange("b one -> (one) (b)"), o2s[:])

    nc.compile()
    return nc


# ---------------------------------------------------------------------------
# host-side packing
# ---------------------------------------------------------------------------

def pack_shared(inp):
    """Pack weight tensors (identical across cores)."""
    f = np.float32
    d = {}
    wih = {0: inp["Wih_l"], 1: inp["Wih_a"], 2: inp["Wih_v"]}
    whh = {0: inp["Whh_l"], 1: inp["Whh_a"], 2: inp["Whh_v"]}
    bb = {m: (inp[f"bih_{k}"] + inp[f"bhh_{k}"]).astype(f)
          for m, k in ((0, "l"), (1, "a"), (2, "v"))}
    foff = {0: 0, 1: D_L, 2: D_L + D_A}
    din = {0: D_L, 1: D_A, 2: D_V}

    waug = np.zeros((512, 1536), f)
    whhT = np.zeros((128, 1536), f)
    for gq in range(4):
        tg = TORCH_G[gq]
        for m in range(3):
            s = gq * 3 + m
            wblk = wih[m][tg * 128:(tg + 1) * 128, :]          # [128, din]
            waug[foff[m]:foff[m] + din[m], s * 128:(s + 1) * 128] = wblk.T
            waug[DIN, s * 128:(s + 1) * 128] = bb[m][tg * 128:(tg + 1) * 128]
            whhT[:, s * 128:(s + 1) * 128] = whh[m][tg * 128:(tg + 1) * 128, :].T
    d["waug"] = waug.astype(NPBF)
    d["whhT"] = whhT.astype(NPBF)
    d["ident"] = np.eye(128, dtype=f).astype(NPBF)
    d["ones128"] = np.ones((128, 1), f).astype(NPBF)

    d["a1w1"] = inp["att1_W1"].T.astype(NPBF).copy()           # [768, 256]
    d["a1b1"] = inp["att1_b1"].reshape(2, 128).T.astype(f).copy()
    d["a1w2"] = inp["att1_W2"].T.astype(NPBF).copy()           # [256, 768]
    d["a1b2"] = inp["att1_b2"].reshape(6, 128).T.astype(f).copy()
    d["a2w1"] = inp["att2_W1"].T.astype(NPBF).copy()
    d["a2b1r"] = inp["att2_b1"].reshape(1, 256).astype(NPBF).copy()
    d["a2w2"] = inp["att2_W2"].T.astype(NPBF).copy()           # [256, 256]
    d["a2b2r"] = inp["att2_b2"].reshape(1, 256).astype(NPBF).copy()
    d["g1a"] = inp["g1_W1"][:, :768].T.astype(NPBF).copy()
    d["g2a"] = inp["g2_W1"][:, :768].T.astype(NPBF).copy()
    d["g1b"] = inp["g1_W1"][:, 768:].T.astype(NPBF).copy()
    d["g2b"] = inp["g2_W1"][:, 768:].T.astype(NPBF).copy()
    d["g1b1r"] = inp["g1_b1"].reshape(1, 256).astype(NPBF).copy()
    d["g2b1r"] = inp["g2_b1"].reshape(1, 256).astype(NPBF).copy()
    d["g1w2"] = inp["g1_W2"].T.astype(NPBF).copy()
    d["g2w2"] = inp["g2_W2"].T.astype(NPBF).copy()
    d["gb2c"] = np.concatenate([inp["g1_b2"], inp["g2_b2"]]).reshape(4, 128).T.astype(f).copy()
    d["ow1"] = inp["out_W1"].T.astype(NPBF).copy()             # [640, 256]
    d["ob1"] = inp["out_b1"].reshape(2, 128).T.astype(f).copy()
    d["ow2"] = inp["out_W2"].T.astype(NPBF).copy()             # [256, 1]
    d["ob2"] = inp["out_b2"].reshape(1, 1).astype(f).copy()
    return d


def pack_x(x, core, Tp):
    """x: [Tp, 256, 409] -> xT [410, Tp*B] bf16 for one core."""
    xc = np.asarray(x[:, core * B:(core + 1) * B, :], np.float32)   # [Tp, B, 409]
    xt = xc.transpose(2, 0, 1).reshape(DIN, Tp * B)
    return np.concatenate([xt, np.ones((1, Tp * B), np.float32)], 0).astype(NPBF)


_CACHE = {}


def _get_program(Tp):
    if Tp not in _CACHE:
        _CACHE[Tp] = build_program(Tp)
    return _CACHE[Tp]


def kernel(**inputs):
    x = np.asarray(inputs["x"])
    Tp = x.shape[0]
    nc = _get_program(Tp)
    shared = pack_shared({k: np.asarray(v) for k, v in inputs.items()})
    in_maps = []
    for c in range(NCORES):
        m = dict(shared)
        m["xT"] = np.ascontiguousarray(pack_x(x, c, Tp))
        in_maps.append(m)
    res = run_bass_kernel_spmd(nc, in_maps, list(range(NCORES))).results
    out = np.concatenate([r["out"] for r in res], axis=0)
    return out.astype(np.float32)


if __name__ == "__main__":
    import time
    t0 = time.time()
    nc = build_program(32)
    print("built in", time.time() - t0, "s")


# revision 7
# speedup vs baseline: 3.8023x; 1.6418x over previous
"""Trainium2 Bass kernel for nn_Contextual_MFN (Memory Fusion Network).

Fused software-pipelined design (per core; batch DP 8 ways, 32 rows/core).
All phases stream through SBUF in CH=8-step windows with stage skew:

  S1(w+1): xWb = Wih_aug @ x (time-parallel, psum->sbuf)
  S2(w):   3xLSTM recurrence step (gates = inject(xwb) + Whh@h)
  S3(w-1): attention: att1 MLP -> exp -> S -> 1/S -> normalized attended
           -> att2/g1/g2 linear parts (+bias via ones-row matmuls)
  S4(w-2): memory-gate recurrence (mem-dependent matmuls only), split into
           A (L1+relu) and B (L2+sigmoid+mem update) half-steps

One superstep = one (w, j) iteration emitting a slice of every stage, so
the serial chains of S2/S4 hide under S1/S3 tensor work. No barriers, no
intermediate DRAM. All matmuls bf16 (FWL), psum fp32.
"""
import numpy as np
import ml_dtypes

import concourse.bass as bass
import concourse.tile as tile
from concourse import bacc, mybir
from concourse.bass_utils import run_bass_kernel_spmd

F32 = mybir.dt.float32
BF = mybir.dt.bfloat16
AF = mybir.ActivationFunctionType
NPBF = ml_dtypes.bfloat16

T_FULL = 512
NBATCH = 256
NCORES = 8
B = NBATCH // NCORES          # 32 batch rows per core
D_L, D_A, D_V = 300, 74, 35
DIN = D_L + D_A + D_V         # 409
DAUG = DIN + 1                # 410 (ones row for bias)
DH = 128
MEM = 256
CH = 8                        # window size (steps)
NB = CH * B                   # 256: window free dim

TORCH_G = (0, 1, 3, 2)        # our slot g' -> torch gate row block


def _nonzero_kcs(s):
    m = s % 3
    if m == 0:
        return [0, 1, 2, 3]
    return [2, 3]


def build_program(Tp=T_FULL):
    assert Tp % CH == 0
    NW = Tp // CH
    nc = bacc.Bacc("TRN2", target_bir_lowering=False, debug=False)

    # ---------------- external inputs ----------------
    xT = nc.dram_tensor("xT", [DAUG, Tp * B], BF, kind="ExternalInput")
    waug = nc.dram_tensor("waug", [512, 1536], BF, kind="ExternalInput")
    whhT = nc.dram_tensor("whhT", [128, 1536], BF, kind="ExternalInput")
    ident = nc.dram_tensor("ident", [128, 128], BF, kind="ExternalInput")
    ones128 = nc.dram_tensor("ones128", [128, 1], BF, kind="ExternalInput")

    a1w1 = nc.dram_tensor("a1w1", [768, 256], BF, kind="ExternalInput")
    a1b1r = nc.dram_tensor("a1b1r", [1, 256], BF, kind="ExternalInput")
    a1w2 = nc.dram_tensor("a1w2", [256, 768], BF, kind="ExternalInput")
    a1b2r = nc.dram_tensor("a1b2r", [1, 768], BF, kind="ExternalInput")
    a2w1 = nc.dram_tensor("a2w1", [768, 256], BF, kind="ExternalInput")
    a2b1r = nc.dram_tensor("a2b1r", [1, 256], BF, kind="ExternalInput")
    a2w2 = nc.dram_tensor("a2w2", [256, 256], BF, kind="ExternalInput")
    a2b2r = nc.dram_tensor("a2b2r", [1, 256], BF, kind="ExternalInput")
    g1a = nc.dram_tensor("g1a", [768, 256], BF, kind="ExternalInput")
    g2a = nc.dram_tensor("g2a", [768, 256], BF, kind="ExternalInput")
    g1b = nc.dram_tensor("g1b", [256, 256], BF, kind="ExternalInput")
    g2b = nc.dram_tensor("g2b", [256, 256], BF, kind="ExternalInput")
    g1b1r = nc.dram_tensor("g1b1r", [1, 256], BF, kind="ExternalInput")
    g2b1r = nc.dram_tensor("g2b1r", [1, 256], BF, kind="ExternalInput")
    g1w2 = nc.dram_tensor("g1w2", [256, 256], BF, kind="ExternalInput")
    g2w2 = nc.dram_tensor("g2w2", [256, 256], BF, kind="ExternalInput")
    gb2c = nc.dram_tensor("gb2c", [128, 4], F32, kind="ExternalInput")
    ow1 = nc.dram_tensor("ow1", [640, 256], BF, kind="ExternalInput")
    ob1 = nc.dram_tensor("ob1", [128, 2], F32, kind="ExternalInput")
    ow2 = nc.dram_tensor("ow2", [256, 1], BF, kind="ExternalInput")
    ob2 = nc.dram_tensor("ob2", [1, 1], F32, kind="ExternalInput")

    out_d = nc.dram_tensor("out", [B, 1], F32, kind="ExternalOutput")

    import contextlib
    with tile.TileContext(nc) as tc:
        ctx = contextlib.ExitStack()
        with ctx:
            wpool = ctx.enter_context(tc.tile_pool(name="weights", bufs=1))
            hpool = ctx.enter_context(tc.tile_pool(name="hstate", bufs=2))
            xtpool = ctx.enter_context(tc.tile_pool(name="xt", bufs=3))
            xwpool = ctx.enter_context(tc.tile_pool(name="xwb", bufs=2))
            cpool = ctx.enter_context(tc.tile_pool(name="cwin", bufs=3))
            s2pool = ctx.enter_context(tc.tile_pool(name="s2tmp", bufs=3))
            s3pool = ctx.enter_context(tc.tile_pool(name="s3tmp", bufs=2))
            gvpool = ctx.enter_context(tc.tile_pool(name="gav", bufs=2))
            s4pool = ctx.enter_context(tc.tile_pool(name="s4tmp", bufs=3))
            mpool = ctx.enter_context(tc.tile_pool(name="mem", bufs=2))
            # PSUM: exactly 8 banks
            pA = ctx.enter_context(tc.tile_pool(name="pA", bufs=3, space="PSUM"))
            pG = ctx.enter_context(tc.tile_pool(name="pG", bufs=2, space="PSUM"))
            pQ = ctx.enter_context(tc.tile_pool(name="pQ", bufs=2, space="PSUM"))
            pS = ctx.enter_context(tc.tile_pool(name="pS", bufs=1, space="PSUM"))

            # ---- resident weights / constants ----
            wihT_t = wpool.tile([128, 4, 1536], BF)
            nc.sync.dma_start(wihT_t[:], waug.ap().rearrange("(kc p) c -> p kc c", p=128))
            whhT_t = wpool.tile([128, 1536], BF)
            nc.sync.dma_start(whhT_t[:], whhT.ap())
            id_t = wpool.tile([128, 128], BF)
            nc.sync.dma_start(id_t[:], ident.ap())
            ones128_t = wpool.tile([128, 1], BF)
            nc.sync.dma_start(ones128_t[:], ones128.ap())

            a1w1_t = wpool.tile([128, 6, 256], BF)
            nc.sync.dma_start(a1w1_t[:], a1w1.ap().rearrange("(kc p) c -> p kc c", p=128))
            a1w2_t = wpool.tile([128, 2, 768], BF)
            nc.sync.dma_start(a1w2_t[:], a1w2.ap().rearrange("(kc p) c -> p kc c", p=128))
            a2w1_t = wpool.tile([128, 6, 256], BF)
            nc.sync.dma_start(a2w1_t[:], a2w1.ap().rearrange("(kc p) c -> p kc c", p=128))
            a2w2_t = wpool.tile([128, 2, 256], BF)
            nc.sync.dma_start(a2w2_t[:], a2w2.ap().rearrange("(kc p) c -> p kc c", p=128))
            g1a_t = wpool.tile([128, 6, 256], BF)
            nc.sync.dma_start(g1a_t[:], g1a.ap().rearrange("(kc p) c -> p kc c", p=128))
            g2a_t = wpool.tile([128, 6, 256], BF)
            nc.sync.dma_start(g2a_t[:], g2a.ap().rearrange("(kc p) c -> p kc c", p=128))
            g1b_t = wpool.tile([128, 2, 256], BF)
            nc.sync.dma_start(g1b_t[:], g1b.ap().rearrange("(kc p) c -> p kc c", p=128))
            g2b_t = wpool.tile([128, 2, 256], BF)
            nc.sync.dma_start(g2b_t[:], g2b.ap().rearrange("(kc p) c -> p kc c", p=128))
            g1w2_t = wpool.tile([128, 2, 256], BF)
            nc.sync.dma_start(g1w2_t[:], g1w2.ap().rearrange("(kc p) c -> p kc c", p=128))
            g2w2_t = wpool.tile([128, 2, 256], BF)
            nc.sync.dma_start(g2w2_t[:], g2w2.ap().rearrange("(kc p) c -> p kc c", p=128))
            a1b1r_t = wpool.tile([1, 256], BF)
            nc.sync.dma_start(a1b1r_t[:], a1b1r.ap())
            a1b2r_t = wpool.tile([1, 768], BF)
            nc.sync.dma_start(a1b2r_t[:], a1b2r.ap())
            a2b1r_t = wpool.tile([1, 256], BF)
            nc.sync.dma_start(a2b1r_t[:], a2b1r.ap())
            a2b2r_t = wpool.tile([1, 256], BF)
            nc.sync.dma_start(a2b2r_t[:], a2b2r.ap())
            g1b1r_t = wpool.tile([1, 256], BF)
            nc.sync.dma_start(g1b1r_t[:], g1b1r.ap())
            g2b1r_t = wpool.tile([1, 256], BF)
            nc.sync.dma_start(g2b1r_t[:], g2b1r.ap())
            gb2_t = wpool.tile([128, 4], F32)
            nc.sync.dma_start(gb2_t[:], gb2c.ap())
            ow1_t = wpool.tile([128, 5, 256], BF)
            nc.sync.dma_start(ow1_t[:], ow1.ap().rearrange("(kc p) c -> p kc c", p=128))
            ob1_t = wpool.tile([128, 2], F32)
            nc.sync.dma_start(ob1_t[:], ob1.ap())
            ow2_t = wpool.tile([128, 2, 1], BF)
            nc.sync.dma_start(ow2_t[:], ow2.ap().rearrange("(kc p) c -> p kc c", p=128))
            ob2_t = wpool.tile([1, 1], F32)
            nc.sync.dma_start(ob2_t[:], ob2.ap())

            ones1x128_t = wpool.tile([1, 128], BF)
            nc.vector.memset(ones1x128_t[:], 1.0)
            ones256_t = wpool.tile([1, 256], BF)
            nc.vector.memset(ones256_t[:], 1.0)

            # ---------------- per-window state ----------------
            xt_tiles = {}       # w -> x input tile [128, 4, NB]
            xwb_tiles = {}      # w -> xWb tile [128, 12, CH, 32]
            cw_tiles = {}       # w -> c window [128, 3, CH+1, 32]
            gav_tiles = {}      # w -> attended-linear outputs [128, 6, NB]
            chw_tiles = {}      # w -> cHat window [128, 2, NB]
            s4_hh = {}          # (w, j) -> relu tile for S4 B-half

            def dma_x(w):
                t = xtpool.tile([128, 4, NB], BF, tag="xt")
                t0 = w * CH
                for kc in range(4):
                    rows = 128 if kc < 3 else DAUG - 384
                    nc.sync.dma_start(
                        t[0:rows, kc, :],
                        xT.ap()[kc * 128:kc * 128 + rows, t0 * B:(t0 + CH) * B])
                xt_tiles[w] = t

            def s1_gen2(w):
                """xWb for window w: 6 chunks (one slot-pair each), then 2 empty."""
                xw = xwpool.tile([128, 12, CH, 32], BF, tag="xwb")
                xwb_tiles[w] = xw
                xt = xt_tiles[w]
                for p in range(6):
                    pt = pA.tile([128, 2, NB], F32, tag="big")
                    for si in range(2):
                        s = 2 * p + si
                        kcs = _nonzero_kcs(s)
                        for i, kc in enumerate(kcs):
                            rows = 128 if kc < 3 else DAUG - 384
                            nc.tensor.matmul(
                                pt[:, si, :], wihT_t[0:rows, kc, s * 128:(s + 1) * 128],
                                xt[0:rows, kc, :],
                                start=(i == 0), stop=(i == len(kcs) - 1))
                    dst = xw[:, 2 * p:2 * p + 2, :, :]
                    src = pt[:].rearrange("p a (t b) -> p a t b", b=32)
                    if p % 2 == 0:
                        nc.vector.tensor_copy(dst, src)
                    else:
                        nc.scalar.copy(dst, src)
                    yield
                for _ in range(2):
                    yield

            h_cur = [None]

            def s2_step(w, j):
                if j == 0:
                    cw = cpool.tile([128, 3, CH + 1, 32], BF, tag="cw")
                    cw_tiles[w] = cw
                    if w == 0:
                        nc.vector.memset(cw[:, :, 0, :], 0.0)
                    else:
                        nc.vector.tensor_copy(cw[:, :, 0, :],
                                              cw_tiles[w - 1][:, :, CH, :])
                cw = cw_tiles[w]
                gp = pG.tile([128, 12, 32], F32, tag="gates")
                nc.tensor.matmul(gp[:], id_t[:], xwb_tiles[w][:, :, j, :],
                                 start=True, stop=False)
                h = h_cur[0]
                for s in range(12):
                    gq, m = divmod(s, 3)
                    nc.tensor.matmul(
                        gp[:, s, :], whhT_t[:, s * 128:(s + 1) * 128],
                        h[:, m * 32:(m + 1) * 32],
                        start=False, stop=(s == 11))
                sg = s2pool.tile([128, 9, 32], BF, tag="sg")
                nc.scalar.activation(sg[:], gp[:, 0:9, :], AF.Sigmoid)
                tg = s2pool.tile([128, 3, 32], BF, tag="tg")
                nc.scalar.activation(tg[:], gp[:, 9:12, :], AF.Tanh)
                t1 = s2pool.tile([128, 3, 32], BF, tag="t1")
                nc.vector.tensor_mul(t1[:], sg[:, 0:3, :], tg[:])
                t2 = s2pool.tile([128, 3, 32], BF, tag="t2")
                nc.vector.tensor_mul(t2[:], sg[:, 3:6, :], cw[:, :, j, :])
                nc.vector.tensor_add(cw[:, :, j + 1, :], t1[:], t2[:])
                return sg, cw

            def s2_tail(w, j, sg, cw):
                tct = s2pool.tile([128, 3, 32], BF, tag="tc")
                nc.scalar.activation(tct[:], cw[:, :, j + 1, :], AF.Tanh)
                h_new = hpool.tile([128, 96], BF, tag="h")
                nc.vector.tensor_mul(
                    h_new[:].rearrange("p (m b) -> p m b", b=32),
                    sg[:, 6:9, :], tct[:])
                h_cur[0] = h_new

            def s3_gen(w):
                """Attention for window w, 8 chunks."""
                cw = cw_tiles[w]

                def rhs_k(kc):
                    if kc < 3:
                        return cw[:, kc, 0:CH, :]
                    return cw[:, kc - 3, 1:CH + 1, :]

                # --- chunk 0: att1 L1 + relu ---
                y1p = pA.tile([128, 2, NB], F32, tag="big")
                for mc in range(2):
                    for kc in range(6):
                        nc.tensor.matmul(
                            y1p[:, mc, :], a1w1_t[:, kc, mc * 128:(mc + 1) * 128],
                            rhs_k(kc), start=(kc == 0), stop=False)
                    nc.tensor.matmul(y1p[:, mc, :], a1b1r_t[:, mc * 128:(mc + 1) * 128],
                                     ones256_t[:], start=False, stop=True)
                y1 = s3pool.tile([128, 2, NB], BF, tag="y1")
                nc.scalar.activation(y1[:], y1p[:], AF.Relu)
                yield
                # --- chunks 2-4: att1 L2 pairs + exp + S partial sums ---
                et = s3pool.tile([128, 6, NB], BF, tag="et")
                sp = pS.tile([1, NB], F32, tag="sp")
                for pr in range(3):
                    ep = pA.tile([128, 2, NB], F32, tag="big")
                    for si in range(2):
                        mc6 = 2 * pr + si
                        for kc in range(2):
                            nc.tensor.matmul(
                                ep[:, si, :], a1w2_t[:, kc, mc6 * 128:(mc6 + 1) * 128],
                                y1[:, kc, :], start=(kc == 0), stop=False)
                        nc.tensor.matmul(ep[:, si, :],
                                         a1b2r_t[:, mc6 * 128:(mc6 + 1) * 128],
                                         ones256_t[:], start=False, stop=True)
                    nc.scalar.activation(et[:, 2 * pr:2 * pr + 2, :], ep[:], AF.Exp)
                    for si in range(2):
                        nc.tensor.matmul(sp[:], ones128_t[:], et[:, 2 * pr + si, :],
                                         start=(pr == 0 and si == 0),
                                         stop=(pr == 2 and si == 1),
                                         skip_group_check=True)
                    yield
                # --- chunk 5: 1/S, broadcast, normalized attended ---
                sinv = s3pool.tile([1, NB], F32, tag="sinvf")
                nc.vector.reciprocal(sinv[:], sp[:])
                sinvb = s3pool.tile([1, NB], BF, tag="sinvb")
                nc.vector.tensor_copy(sinvb[:], sinv[:])
                sbp = pA.tile([128, 2, NB], F32, tag="big")
                nc.tensor.matmul(sbp[:, 0, :], ones1x128_t[:], sinvb[:],
                                 start=True, stop=True)
                sb_w = s3pool.tile([128, NB], BF, tag="sbw")
                nc.scalar.copy(sb_w[:], sbp[:, 0, :])
                cn = s3pool.tile([128, 6, NB], BF, tag="cn")
                sbv = sb_w[:].rearrange("p (t b) -> p t b", b=32)
                nc.vector.tensor_mul(
                    cn[:, 0:3, :].rearrange("p m (t b) -> p m t b", b=32),
                    cw[:, :, 0:CH, :],
                    sbv.unsqueeze(1).broadcast_to([128, 3, CH, 32]))
                nc.vector.tensor_mul(
                    cn[:, 3:6, :].rearrange("p m (t b) -> p m t b", b=32),
                    cw[:, :, 1:CH + 1, :],
                    sbv.unsqueeze(1).broadcast_to([128, 3, CH, 32]))
                yield
                # --- chunk 6: utn; att2 L1 + relu ---
                utn = s3pool.tile([128, 6, NB], BF, tag="utn")
                nc.vector.tensor_mul(utn[:], et[:], cn[:])
                zp = pA.tile([128, 2, NB], F32, tag="big")
                for mc in range(2):
                    for kc in range(6):
                        nc.tensor.matmul(
                            zp[:, mc, :], a2w1_t[:, kc, mc * 128:(mc + 1) * 128],
                            utn[:, kc, :], start=(kc == 0), stop=False)
                    nc.tensor.matmul(zp[:, mc, :], a2b1r_t[:, mc * 128:(mc + 1) * 128],
                                     ones256_t[:], start=False, stop=True)
                z = s3pool.tile([128, 2, NB], BF, tag="z")
                nc.scalar.activation(z[:], zp[:], AF.Relu)
                yield
                # --- chunk 7a: att2 L2 + cHat tanh; g1 ---
                gav = gvpool.tile([128, 6, NB], BF, tag="gav")
                gav_tiles[w] = gav
                ap2 = pA.tile([128, 2, NB], F32, tag="big")
                for mc in range(2):
                    for kc in range(2):
                        nc.tensor.matmul(
                            ap2[:, mc, :], a2w2_t[:, kc, mc * 128:(mc + 1) * 128],
                            z[:, kc, :], start=(kc == 0), stop=False)
                    nc.tensor.matmul(ap2[:, mc, :], a2b2r_t[:, mc * 128:(mc + 1) * 128],
                                     ones256_t[:], start=False, stop=True)
                chw = gvpool.tile([128, 2, NB], BF, tag="chw")
                chw_tiles[w] = chw
                nc.scalar.activation(chw[:], ap2[:], AF.Tanh)
                g1p = pA.tile([128, 2, NB], F32, tag="big")
                for mc in range(2):
                    for kc in range(6):
                        nc.tensor.matmul(
                            g1p[:, mc, :], g1a_t[:, kc, mc * 128:(mc + 1) * 128],
                            utn[:, kc, :], start=(kc == 0), stop=False)
                    nc.tensor.matmul(g1p[:, mc, :], g1b1r_t[:, mc * 128:(mc + 1) * 128],
                                     ones256_t[:], start=False, stop=True)
                nc.vector.tensor_copy(gav[:, 0:2, :], g1p[:])
                yield
                # --- chunk 7b: g2 ---
                g2p = pA.tile([128, 2, NB], F32, tag="big")
                for mc in range(2):
                    for kc in range(6):
                        nc.tensor.matmul(
                            g2p[:, mc, :], g2a_t[:, kc, mc * 128:(mc + 1) * 128],
                            utn[:, kc, :], start=(kc == 0), stop=False)
                    nc.tensor.matmul(g2p[:, mc, :], g2b1r_t[:, mc * 128:(mc + 1) * 128],
                                     ones256_t[:], start=False, stop=True)
                nc.vector.tensor_copy(gav[:, 2:4, :], g2p[:])
                yield

            mem_cur = [None]

            def s4A(w, j):
                pg = pQ.tile([128, 4, 32], F32, tag="pq")
                mem = mem_cur[0]
                for r in range(4):
                    mc = r % 2
                    gwt = g1b_t if r < 2 else g2b_t
                    for kc in range(2):
                        nc.tensor.matmul(
                            pg[:, r, :], gwt[:, kc, mc * 128:(mc + 1) * 128],
                            mem[:, kc, :], start=(kc == 0), stop=(kc == 1))
                w_t = s4pool.tile([128, 4, 32], BF, tag="w")
                gav = gav_tiles[w]
                nc.vector.tensor_add(
                    w_t[:], gav[:, 0:4, j * 32:(j + 1) * 32], pg[:])
                hh = s4pool.tile([128, 4, 32], BF, tag="hh")
                nc.scalar.activation(hh[:], w_t[:], AF.Relu)
                s4_hh[(w, j)] = (hh, mem)

            def s4B(w, j):
                hh, mem = s4_hh.pop((w, j))
                qg = pQ.tile([128, 4, 32], F32, tag="pq")
                for r in range(4):
                    mc = r % 2
                    goff = 0 if r < 2 else 2
                    gwt = g1w2_t if r < 2 else g2w2_t
                    for kc in range(2):
                        nc.tensor.matmul(
                            qg[:, r, :], gwt[:, kc, mc * 128:(mc + 1) * 128],
                            hh[:, goff + kc, :], start=(kc == 0), stop=(kc == 1))
                gpre = s4pool.tile([128, 4, 32], F32, tag="gpre")
                nc.vector.tensor_add(
                    gpre[:], qg[:],
                    gb2_t[:].unsqueeze(2).broadcast_to([128, 4, 32]))
                gam = s4pool.tile([128, 4, 32], BF, tag="gam")
                nc.scalar.activation(gam[:], gpre[:], AF.Sigmoid)
                m1 = s4pool.tile([128, 2, 32], BF, tag="m1")
                nc.vector.tensor_mul(m1[:], gam[:, 0:2, :], mem[:])
                m2 = s4pool.tile([128, 2, 32], BF, tag="m2")
                chw = chw_tiles[w]
                nc.vector.tensor_mul(
                    m2[:], gam[:, 2:4, :],
                    chw[:].rearrange("p a (t b) -> p a t b", b=32)[:, :, j, :])
                mem_new = mpool.tile([128, 2, 32], BF, tag="mem")
                nc.vector.tensor_add(mem_new[:], m1[:], m2[:])
                mem_cur[0] = mem_new

            # ---------------- prologue ----------------
            h0 = hpool.tile([128, 96], BF, tag="h")
            nc.vector.memset(h0[:], 0.0)
            h_cur[0] = h0
            m0 = mpool.tile([128, 2, 32], BF, tag="mem")
            nc.vector.memset(m0[:], 0.0)
            mem_cur[0] = m0

            dma_x(0)
            if NW > 1:
                dma_x(1)
            # full S1 for window 0 as a warmup burst
            for _ in s1_gen2(0):
                pass

            s1_iters = {}
            s3_iters = {}
            pending_B = [None]

            # ---------------- main superstep loop ----------------
            for w in range(NW + 2):
                if w + 1 < NW:
                    s1_iters[w + 1] = s1_gen2(w + 1)
                if 1 <= w <= NW:
                    s3_iters[w - 1] = s3_gen(w - 1)
                if w + 2 < NW:
                    dma_x(w + 2)
                for j in range(CH):
                    if w < NW:
                        sg, cwt = s2_step(w, j)
                    if w + 1 < NW:
                        next(s1_iters[w + 1], None)
                    if w < NW:
                        s2_tail(w, j, sg, cwt)
                    if 1 <= w <= NW:
                        next(s3_iters[w - 1], None)
                    if 2 <= w <= NW + 1:
                        if pending_B[0] is not None:
                            s4B(*pending_B[0])
                        s4A(w - 2, j)
                        pending_B[0] = (w - 2, j)
            s4B(*pending_B[0])

            # ---------------- output MLP ----------------
            h_fin = h_cur[0]
            mem = mem_cur[0]
            o1p = pQ.tile([128, 4, 32], F32, tag="pq")
            rhs5 = [h_fin[:, 0:32], h_fin[:, 32:64], h_fin[:, 64:96],
                    mem[:, 0, :], mem[:, 1, :]]
            for mc in range(2):
                for kc in range(5):
                    nc.tensor.matmul(
                        o1p[:, mc, :], ow1_t[:, kc, mc * 128:(mc + 1) * 128],
                        rhs5[kc], start=(kc == 0), stop=(kc == 4))
            o1s = s4pool.tile([128, 2, 32], BF, tag="o1s")
            for mc in range(2):
                nc.scalar.activation(o1s[:, mc, :], o1p[:, mc, :], AF.Relu,
                                     bias=ob1_t[:, mc:mc + 1])
            o2p = pS.tile([1, NB], F32, tag="sp")
            for kc in range(2):
                nc.tensor.matmul(o2p[:, 0:32], ow2_t[:, kc, :], o1s[:, kc, :],
                                 start=(kc == 0), stop=(kc == 1))
            o2s = s4pool.tile([1, 32], F32, tag="o2s")
            nc.scalar.activation(o2s[:], o2p[:, 0:32], AF.Identity, bias=ob2_t[:])
            nc.sync.dma_start(out_d.ap().rearrange("b one -> (one) (b)"), o2s[:])

    nc.compile()
    return nc


# ---------------------------------------------------------------------------
# host-side packing
# ---------------------------------------------------------------------------

def pack_shared(inp):
    f = np.float32
    d = {}
    wih = {0: inp["Wih_l"], 1: inp["Wih_a"], 2: inp["Wih_v"]}
    whh = {0: inp["Whh_l"], 1: inp["Whh_a"], 2: inp["Whh_v"]}
    bb = {m: (inp[f"bih_{k}"] + inp[f"bhh_{k}"]).astype(f)
          for m, k in ((0, "l"), (1, "a"), (2, "v"))}
    foff = {0: 0, 1: D_L, 2: D_L + D_A}
    din = {0: D_L, 1: D_A, 2: D_V}

    waug = np.zeros((512, 1536), f)
    whhT = np.zeros((128, 1536), f)
    for gq in range(4):
        tg = TORCH_G[gq]
        for m in range(3):
            s = gq * 3 + m
            wblk = wih[m][tg * 128:(tg + 1) * 128, :]
            waug[foff[m]:foff[m] + din[m], s * 128:(s + 1) * 128] = wblk.T
            waug[DIN, s * 128:(s + 1) * 128] = bb[m][tg * 128:(tg + 1) * 128]
            whhT[:, s * 128:(s + 1) * 128] = whh[m][tg * 128:(tg + 1) * 128, :].T
    d["waug"] = waug.astype(NPBF)
    d["whhT"] = whhT.astype(NPBF)
    d["ident"] = np.eye(128, dtype=f).astype(NPBF)
    d["ones128"] = np.ones((128, 1), f).astype(NPBF)

    d["a1w1"] = inp["att1_W1"].T.astype(NPBF).copy()
    d["a1b1r"] = inp["att1_b1"].reshape(1, 256).astype(NPBF).copy()
    d["a1w2"] = inp["att1_W2"].T.astype(NPBF).copy()
    d["a1b2r"] = inp["att1_b2"].reshape(1, 768).astype(NPBF).copy()
    d["a2w1"] = inp["att2_W1"].T.astype(NPBF).copy()
    d["a2b1r"] = inp["att2_b1"].reshape(1, 256).astype(NPBF).copy()
    d["a2w2"] = inp["att2_W2"].T.astype(NPBF).copy()
    d["a2b2r"] = inp["att2_b2"].reshape(1, 256).astype(NPBF).copy()
    d["g1a"] = inp["g1_W1"][:, :768].T.astype(NPBF).copy()
    d["g2a"] = inp["g2_W1"][:, :768].T.astype(NPBF).copy()
    d["g1b"] = inp["g1_W1"][:, 768:].T.astype(NPBF).copy()
    d["g2b"] = inp["g2_W1"][:, 768:].T.astype(NPBF).copy()
    d["g1b1r"] = inp["g1_b1"].reshape(1, 256).astype(NPBF).copy()
    d["g2b1r"] = inp["g2_b1"].reshape(1, 256).astype(NPBF).copy()
    d["g1w2"] = inp["g1_W2"].T.astype(NPBF).copy()
    d["g2w2"] = inp["g2_W2"].T.astype(NPBF).copy()
    d["gb2c"] = np.concatenate([inp["g1_b2"], inp["g2_b2"]]).reshape(4, 128).T.astype(f).copy()
    d["ow1"] = inp["out_W1"].T.astype(NPBF).copy()
    d["ob1"] = inp["out_b1"].reshape(2, 128).T.astype(f).copy()
    d["ow2"] = inp["out_W2"].T.astype(NPBF).copy()
    d["ob2"] = inp["out_b2"].reshape(1, 1).astype(f).copy()
    return d


def pack_x(x, core, Tp):
    xc = np.asarray(x[:, core * B:(core + 1) * B, :], np.float32)
    xt = xc.transpose(2, 0, 1).reshape(DIN, Tp * B)
    return np.concatenate([xt, np.ones((1, Tp * B), np.float32)], 0).astype(NPBF)


_CACHE = {}


def _get_program(Tp):
    if Tp not in _CACHE:
        _CACHE[Tp] = build_program(Tp)
    return _CACHE[Tp]


def kernel(**inputs):
    x = np.asarray(inputs["x"])
    Tp = x.shape[0]
    nc = _get_program(Tp)
    shared = pack_shared({k: np.asarray(v) for k, v in inputs.items()})
    in_maps = []
    for c in range(NCORES):
        m = dict(shared)
        m["xT"] = np.ascontiguousarray(pack_x(x, c, Tp))
        in_maps.append(m)
    res = run_bass_kernel_spmd(nc, in_maps, list(range(NCORES))).results
    out = np.concatenate([r["out"] for r in res], axis=0)
    return out.astype(np.float32)


if __name__ == "__main__":
    import time
    t0 = time.time()
    nc = build_program(64)
    print("built in", time.time() - t0, "s")


# revision 21
# speedup vs baseline: 3.9530x; 1.0396x over previous
"""Trainium2 Bass kernel for nn_Contextual_MFN (Memory Fusion Network).

Fused software-pipelined design (per core; batch DP 8 ways, 32 rows/core).
All phases stream through SBUF in CH=8-step windows with stage skew:

  S1(w+1): xWb = Wih_aug @ x (time-parallel, psum->sbuf)
  S2(w):   3xLSTM recurrence step (gates = inject(xwb) + Whh@h)
  S3(w-1): attention: att1 MLP -> exp -> S -> 1/S -> normalized attended
           -> att2/g1/g2 linear parts (+bias via ones-row matmuls)
  S4(w-2): memory-gate recurrence (mem-dependent matmuls only), split into
           A (L1+relu) and B (L2+sigmoid+mem update) half-steps

One superstep = one (w, j) iteration emitting a slice of every stage, so
the serial chains of S2/S4 hide under S1/S3 tensor work. No barriers, no
intermediate DRAM. All matmuls bf16 (FWL), psum fp32.
"""
import numpy as np
import ml_dtypes

import concourse.bass as bass
import concourse.tile as tile
from concourse import bacc, mybir
from concourse.bass_utils import run_bass_kernel_spmd

F32 = mybir.dt.float32
BF = mybir.dt.bfloat16
AF = mybir.ActivationFunctionType
NPBF = ml_dtypes.bfloat16

T_FULL = 512
NBATCH = 256
NCORES = 8
B = NBATCH // NCORES          # 32 batch rows per core
D_L, D_A, D_V = 300, 74, 35
DIN = D_L + D_A + D_V         # 409
DAUG = DIN + 1                # 410 (ones row for bias)
DH = 128
MEM = 256
CH = 8                        # window size (steps)
NB = CH * B                   # 256: window free dim

TORCH_G = (0, 1, 3, 2)        # our slot g' -> torch gate row block

# x / Waug row packing (512 rows, 4 K-chunks of 128):
#   kc0: l[0:128];  kc1: l[128:256]
#   kc2: rows 0:45 = l[256:300]+bias_l ; rows 64:100 = v[0:35]+bias_v
#   kc3: rows 0:75 = a[0:74]+bias_a
# -> per-slot matmul specs (kc, row_lo, row_hi); partition bases 0/64 only.
MM_SPECS = {
    0: [(0, 0, 128), (1, 0, 128), (2, 0, 45)],   # modality l
    1: [(3, 0, 75)],                             # modality a
    2: [(2, 64, 100)],                           # modality v
}


def build_program(Tp=T_FULL):
    assert Tp % CH == 0
    NW = Tp // CH
    nc = bacc.Bacc("TRN2", target_bir_lowering=False, debug=False)

    # ---------------- external inputs ----------------
    xT = nc.dram_tensor("xT", [512, Tp * B], BF, kind="ExternalInput")
    waug = nc.dram_tensor("waug", [512, 1536], BF, kind="ExternalInput")
    whhT = nc.dram_tensor("whhT", [128, 1536], BF, kind="ExternalInput")
    ident = nc.dram_tensor("ident", [128, 128], BF, kind="ExternalInput")
    ones128 = nc.dram_tensor("ones128", [128, 1], BF, kind="ExternalInput")

    a1w1 = nc.dram_tensor("a1w1", [768, 256], BF, kind="ExternalInput")
    a1b1c = nc.dram_tensor("a1b1c", [128, 2], F32, kind="ExternalInput")
    a1w2 = nc.dram_tensor("a1w2", [256, 768], BF, kind="ExternalInput")
    a1b2c = nc.dram_tensor("a1b2c", [128, 6], F32, kind="ExternalInput")
    a2w1 = nc.dram_tensor("a2w1", [768, 256], BF, kind="ExternalInput")
    a2b1c = nc.dram_tensor("a2b1c", [128, 2], F32, kind="ExternalInput")
    a2w2 = nc.dram_tensor("a2w2", [256, 256], BF, kind="ExternalInput")
    a2b2c = nc.dram_tensor("a2b2c", [128, 2], F32, kind="ExternalInput")
    g1a = nc.dram_tensor("g1a", [768, 256], BF, kind="ExternalInput")
    g2a = nc.dram_tensor("g2a", [768, 256], BF, kind="ExternalInput")
    g1b = nc.dram_tensor("g1b", [256, 256], BF, kind="ExternalInput")
    g2b = nc.dram_tensor("g2b", [256, 256], BF, kind="ExternalInput")
    g1b1c = nc.dram_tensor("g1b1c", [128, 2], F32, kind="ExternalInput")
    g2b1c = nc.dram_tensor("g2b1c", [128, 2], F32, kind="ExternalInput")
    g1w2 = nc.dram_tensor("g1w2", [256, 256], BF, kind="ExternalInput")
    g2w2 = nc.dram_tensor("g2w2", [256, 256], BF, kind="ExternalInput")
    gb2c = nc.dram_tensor("gb2c", [128, 4], F32, kind="ExternalInput")
    ow1 = nc.dram_tensor("ow1", [640, 256], BF, kind="ExternalInput")
    ob1 = nc.dram_tensor("ob1", [128, 2], F32, kind="ExternalInput")
    ow2 = nc.dram_tensor("ow2", [256, 1], BF, kind="ExternalInput")
    ob2 = nc.dram_tensor("ob2", [1, 1], F32, kind="ExternalInput")

    out_d = nc.dram_tensor("out", [B, 1], F32, kind="ExternalOutput")

    import contextlib
    with tile.TileContext(nc) as tc:
        ctx = contextlib.ExitStack()
        with ctx:
            wpool = ctx.enter_context(tc.tile_pool(name="weights", bufs=1))
            hpool = ctx.enter_context(tc.tile_pool(name="hstate", bufs=2))
            xtpool = ctx.enter_context(tc.tile_pool(name="xt", bufs=3))
            xwpool = ctx.enter_context(tc.tile_pool(name="xwb", bufs=2))
            cpool = ctx.enter_context(tc.tile_pool(name="cwin", bufs=3))
            s2pool = ctx.enter_context(tc.tile_pool(name="s2tmp", bufs=3))
            s3pool = ctx.enter_context(tc.tile_pool(name="s3tmp", bufs=2))
            gvpool = ctx.enter_context(tc.tile_pool(name="gav", bufs=2))
            s4pool = ctx.enter_context(tc.tile_pool(name="s4tmp", bufs=3))
            mpool = ctx.enter_context(tc.tile_pool(name="mem", bufs=2))
            # PSUM: exactly 8 banks (pA 2 + pE 3 + pG 1 + pQ 2)
            pA = ctx.enter_context(tc.tile_pool(name="pA", bufs=2, space="PSUM"))
            pE = ctx.enter_context(tc.tile_pool(name="pE", bufs=3, space="PSUM"))
            pG = ctx.enter_context(tc.tile_pool(name="pG", bufs=1, space="PSUM"))
            pQ = ctx.enter_context(tc.tile_pool(name="pQ", bufs=2, space="PSUM"))

            # ---- resident weights / constants ----
            wihT_t = wpool.tile([128, 4, 1536], BF)
            nc.sync.dma_start(wihT_t[:], waug.ap().rearrange("(kc p) c -> p kc c", p=128))
            whhT_t = wpool.tile([128, 1536], BF)
            nc.sync.dma_start(whhT_t[:], whhT.ap())
            id_t = wpool.tile([128, 128], BF)
            nc.sync.dma_start(id_t[:], ident.ap())
            ones128_t = wpool.tile([128, 1], BF)
            nc.sync.dma_start(ones128_t[:], ones128.ap())

            a1w1_t = wpool.tile([128, 6, 256], BF)
            nc.sync.dma_start(a1w1_t[:], a1w1.ap().rearrange("(kc p) c -> p kc c", p=128))
            a1w2_t = wpool.tile([128, 2, 768], BF)
            nc.sync.dma_start(a1w2_t[:], a1w2.ap().rearrange("(kc p) c -> p kc c", p=128))
            a2w1_t = wpool.tile([128, 6, 256], BF)
            nc.sync.dma_start(a2w1_t[:], a2w1.ap().rearrange("(kc p) c -> p kc c", p=128))
            a2w2_t = wpool.tile([128, 2, 256], BF)
            nc.sync.dma_start(a2w2_t[:], a2w2.ap().rearrange("(kc p) c -> p kc c", p=128))
            g1a_t = wpool.tile([128, 6, 256], BF)
            nc.sync.dma_start(g1a_t[:], g1a.ap().rearrange("(kc p) c -> p kc c", p=128))
            g2a_t = wpool.tile([128, 6, 256], BF)
            nc.sync.dma_start(g2a_t[:], g2a.ap().rearrange("(kc p) c -> p kc c", p=128))
            g1b_t = wpool.tile([128, 2, 256], BF)
            nc.sync.dma_start(g1b_t[:], g1b.ap().rearrange("(kc p) c -> p kc c", p=128))
            g2b_t = wpool.tile([128, 2, 256], BF)
            nc.sync.dma_start(g2b_t[:], g2b.ap().rearrange("(kc p) c -> p kc c", p=128))
            g1w2_t = wpool.tile([128, 2, 256], BF)
            nc.sync.dma_start(g1w2_t[:], g1w2.ap().rearrange("(kc p) c -> p kc c", p=128))
            g2w2_t = wpool.tile([128, 2, 256], BF)
            nc.sync.dma_start(g2w2_t[:], g2w2.ap().rearrange("(kc p) c -> p kc c", p=128))
            a1b1c_t = wpool.tile([128, 2], F32)
            nc.sync.dma_start(a1b1c_t[:], a1b1c.ap())
            a1b2c_t = wpool.tile([128, 6], F32)
            nc.sync.dma_start(a1b2c_t[:], a1b2c.ap())
            a2b1c_t = wpool.tile([128, 2], F32)
            nc.sync.dma_start(a2b1c_t[:], a2b1c.ap())
            a2b2c_t = wpool.tile([128, 2], F32)
            nc.sync.dma_start(a2b2c_t[:], a2b2c.ap())
            g1b1c_t = wpool.tile([128, 2], F32)
            nc.sync.dma_start(g1b1c_t[:], g1b1c.ap())
            g2b1c_t = wpool.tile([128, 2], F32)
            nc.sync.dma_start(g2b1c_t[:], g2b1c.ap())
            gb2_t = wpool.tile([128, 4], F32)
            nc.sync.dma_start(gb2_t[:], gb2c.ap())
            ow1_t = wpool.tile([128, 5, 256], BF)
            nc.sync.dma_start(ow1_t[:], ow1.ap().rearrange("(kc p) c -> p kc c", p=128))
            ob1_t = wpool.tile([128, 2], F32)
            nc.sync.dma_start(ob1_t[:], ob1.ap())
            ow2_t = wpool.tile([128, 2, 1], BF)
            nc.sync.dma_start(ow2_t[:], ow2.ap().rearrange("(kc p) c -> p kc c", p=128))
            ob2_t = wpool.tile([1, 1], F32)
            nc.sync.dma_start(ob2_t[:], ob2.ap())

            ones1x128_t = wpool.tile([1, 128], BF)
            nc.vector.memset(ones1x128_t[:], 1.0)
            zero256_t = wpool.tile([128, 256], BF)
            nc.vector.memset(zero256_t[:], 0.0)
            OP = mybir.AluOpType

            # ---------------- per-window state ----------------
            xt_tiles = {}       # w -> x input tile [128, 4, NB]
            xwb_tiles = {}      # w -> xWb tile [128, 12, CH, 32]
            cw_tiles = {}       # w -> c window [128, 3, CH+1, 32]
            gav_tiles = {}      # w -> attended-linear outputs [128, 6, NB]
            chw_tiles = {}      # w -> cHat window [128, 2, NB]
            s4_hh = {}          # (w, j) -> relu tile for S4 B-half

            def dma_x(w):
                t = xtpool.tile([128, 4, NB], BF, tag="xt")
                t0 = w * CH
                for kc in range(4):
                    nc.sync.dma_start(
                        t[:, kc, :],
                        xT.ap()[kc * 128:(kc + 1) * 128, t0 * B:(t0 + CH) * B])
                xt_tiles[w] = t

            def s1_gen2(w):
                """xWb for window w: 6 chunks (one slot-pair each), then 2 empty."""
                xw = xwpool.tile([128, 12, NB], BF, tag="xwb")
                xwb_tiles[w] = xw
                xt = xt_tiles[w]
                for p in range(6):
                    pt = pA.tile([128, 2, NB], F32, tag="big")
                    for si in range(2):
                        s = 2 * p + si
                        specs = MM_SPECS[s % 3]
                        for i, (kc, lo, hi) in enumerate(specs):
                            nc.tensor.matmul(
                                pt[:, si, :], wihT_t[lo:hi, kc, s * 128:(s + 1) * 128],
                                xt[lo:hi, kc, :],
                                start=(i == 0), stop=(i == len(specs) - 1))
                    if p % 2 == 0:
                        nc.vector.tensor_copy(xw[:, 2 * p:2 * p + 2, :], pt[:])
                    else:
                        nc.scalar.copy(xw[:, 2 * p:2 * p + 2, :], pt[:])
                    yield
                for _ in range(2):
                    yield

            h_cur = [None]

            def s2_step(w, j):
                if j == 0:
                    cw = cpool.tile([128, 3, CH + 1, 32], BF, tag="cw")
                    cw_tiles[w] = cw
                    if w == 0:
                        nc.vector.memset(cw[:, :, 0, :], 0.0)
                    else:
                        nc.vector.tensor_copy(cw[:, :, 0, :],
                                              cw_tiles[w - 1][:, :, CH, :])
                cw = cw_tiles[w]
                gp = pG.tile([128, 12, 32], F32, tag="gates")
                nc.tensor.matmul(gp[:], id_t[:],
                                 xwb_tiles[w][:, :, j * 32:(j + 1) * 32],
                                 start=True, stop=False)
                h = h_cur[0]
                for s in range(12):
                    gq, m = divmod(s, 3)
                    nc.tensor.matmul(
                        gp[:, s, :], whhT_t[:, s * 128:(s + 1) * 128],
                        h[:, m * 32:(m + 1) * 32],
                        start=False, stop=(s == 11))
                sg = s2pool.tile([128, 9, 32], BF, tag="sg")
                nc.scalar.activation(sg[:], gp[:, 0:9, :], AF.Sigmoid)
                tg = s2pool.tile([128, 3, 32], BF, tag="tg")
                nc.scalar.activation(tg[:], gp[:, 9:12, :], AF.Tanh)
                t1 = s2pool.tile([128, 3, 32], BF, tag="t1")
                nc.vector.tensor_mul(t1[:], sg[:, 0:3, :], tg[:])
                t2 = s2pool.tile([128, 3, 32], BF, tag="t2")
                nc.vector.tensor_mul(t2[:], sg[:, 3:6, :], cw[:, :, j, :])
                nc.vector.tensor_add(cw[:, :, j + 1, :], t1[:], t2[:])
                return sg, cw

            def s2_tail(w, j, sg, cw):
                tct = s2pool.tile([128, 3, 32], BF, tag="tc")
                nc.scalar.activation(tct[:], cw[:, :, j + 1, :], AF.Tanh)
                h_new = hpool.tile([128, 96], BF, tag="h")
                nc.vector.tensor_mul(
                    h_new[:].rearrange("p (m b) -> p m b", b=32),
                    sg[:, 6:9, :], tct[:])
                h_cur[0] = h_new

            def s3_gen(w):
                """Attention for window w, 8 chunks."""
                cw = cw_tiles[w]

                def rhs_k(kc):
                    if kc < 3:
                        return cw[:, kc, 0:CH, :]
                    return cw[:, kc - 3, 1:CH + 1, :]

                # --- chunk 0: att1 L1; relu+bias via DVE stt ---
                y1p = pA.tile([128, 2, NB], F32, tag="big")
                for mc in range(2):
                    for kc in range(6):
                        nc.tensor.matmul(
                            y1p[:, mc, :], a1w1_t[:, kc, mc * 128:(mc + 1) * 128],
                            rhs_k(kc), start=(kc == 0), stop=(kc == 5))
                y1 = s3pool.tile([128, 2, NB], BF, tag="y1")
                for mc in range(2):
                    nc.vector.scalar_tensor_tensor(
                        y1[:, mc, :], y1p[:, mc, :], a1b1c_t[:, mc:mc + 1],
                        zero256_t[:], OP.add, OP.max)
                yield
                # --- chunks 1-3: att1 L2 pairs (exps clustered in chunk 3) ---
                et = s3pool.tile([128, 6, NB], BF, tag="et")
                eps = []
                for pr in range(3):
                    ep = pE.tile([128, 2, NB], F32, tag="ep")
                    eps.append(ep)
                    for si in range(2):
                        mc6 = 2 * pr + si
                        for kc in range(2):
                            nc.tensor.matmul(
                                ep[:, si, :], a1w2_t[:, kc, mc6 * 128:(mc6 + 1) * 128],
                                y1[:, kc, :], start=(kc == 0), stop=(kc == 1))
                    if pr == 2:
                        for q in range(6):
                            nc.scalar.activation(et[:, q, :], eps[q // 2][:, q % 2, :],
                                                 AF.Exp, bias=a1b2c_t[:, q:q + 1])
                    yield
                # --- chunk 4: S sums, 1/S, broadcast, normalized cStar ---
                sp = pE.tile([1, NB], F32, tag="ep")
                for q in range(6):
                    nc.tensor.matmul(sp[:], ones128_t[:], et[:, q, :],
                                     start=(q == 0), stop=(q == 5),
                                     skip_group_check=True)
                sinv = s3pool.tile([1, NB], F32, tag="sinvf")
                nc.vector.reciprocal_approx_fast(sinv[:], sp[:])
                sinvb = s3pool.tile([1, NB], BF, tag="sinvb")
                nc.vector.tensor_copy(sinvb[:], sinv[:])
                sbp = pA.tile([128, 2, NB], F32, tag="big")
                nc.tensor.matmul(sbp[:, 0, :], ones1x128_t[:], sinvb[:],
                                 start=True, stop=True)
                sb_w = s3pool.tile([128, NB], BF, tag="sbw")
                nc.scalar.copy(sb_w[:], sbp[:, 0, :])
                cn = s3pool.tile([128, 6, NB], BF, tag="cn")
                sbv = sb_w[:].rearrange("p (t b) -> p t b", b=32)
                nc.vector.tensor_mul(
                    cn[:, 0:3, :].rearrange("p m (t b) -> p m t b", b=32),
                    cw[:, :, 0:CH, :],
                    sbv.unsqueeze(1).broadcast_to([128, 3, CH, 32]))
                nc.vector.tensor_mul(
                    cn[:, 3:6, :].rearrange("p m (t b) -> p m t b", b=32),
                    cw[:, :, 1:CH + 1, :],
                    sbv.unsqueeze(1).broadcast_to([128, 3, CH, 32]))
                yield
                # --- chunk 5: utn; att2 L1; relu+bias via stt ---
                utn = s3pool.tile([128, 6, NB], BF, tag="utn")
                nc.vector.tensor_mul(utn[:], et[:], cn[:])
                zp = pA.tile([128, 2, NB], F32, tag="big")
                for mc in range(2):
                    for kc in range(6):
                        nc.tensor.matmul(
                            zp[:, mc, :], a2w1_t[:, kc, mc * 128:(mc + 1) * 128],
                            utn[:, kc, :], start=(kc == 0), stop=(kc == 5))
                z = s3pool.tile([128, 2, NB], BF, tag="z")
                for mc in range(2):
                    nc.vector.scalar_tensor_tensor(
                        z[:, mc, :], zp[:, mc, :], a2b1c_t[:, mc:mc + 1],
                        zero256_t[:], OP.add, OP.max)
                yield
                # --- chunk 6: att2 L2 + cHat tanh(x+b2); g1 (+b1 via stt) ---
                gav = gvpool.tile([128, 6, NB], BF, tag="gav")
                gav_tiles[w] = gav
                ap2 = pA.tile([128, 2, NB], F32, tag="big")
                for mc in range(2):
                    for kc in range(2):
                        nc.tensor.matmul(
                            ap2[:, mc, :], a2w2_t[:, kc, mc * 128:(mc + 1) * 128],
                            z[:, kc, :], start=(kc == 0), stop=(kc == 1))
                chw = gvpool.tile([128, 2, NB], BF, tag="chw")
                chw_tiles[w] = chw
                for mc in range(2):
                    nc.scalar.activation(chw[:, mc, :], ap2[:, mc, :], AF.Tanh,
                                         bias=a2b2c_t[:, mc:mc + 1])
                g1p = pA.tile([128, 2, NB], F32, tag="big")
                for mc in range(2):
                    for kc in range(6):
                        nc.tensor.matmul(
                            g1p[:, mc, :], g1a_t[:, kc, mc * 128:(mc + 1) * 128],
                            utn[:, kc, :], start=(kc == 0), stop=(kc == 5))
                for mc in range(2):
                    nc.vector.scalar_tensor_tensor(
                        gav[:, mc, :], g1p[:, mc, :], g1b1c_t[:, mc:mc + 1],
                        zero256_t[:], OP.add, OP.add)
                yield
                # --- chunk 7: g2 (+b1 via stt) ---
                g2p = pA.tile([128, 2, NB], F32, tag="big")
                for mc in range(2):
                    for kc in range(6):
                        nc.tensor.matmul(
                            g2p[:, mc, :], g2a_t[:, kc, mc * 128:(mc + 1) * 128],
                            utn[:, kc, :], start=(kc == 0), stop=(kc == 5))
                for mc in range(2):
                    nc.vector.scalar_tensor_tensor(
                        gav[:, 2 + mc, :], g2p[:, mc, :], g2b1c_t[:, mc:mc + 1],
                        zero256_t[:], OP.add, OP.add)
                yield

            mem_cur = [None]

            def s4A(w, j):
                pg = pQ.tile([128, 4, 32], F32, tag="pq")
                mem = mem_cur[0]
                for r in range(4):
                    mc = r % 2
                    gwt = g1b_t if r < 2 else g2b_t
                    for kc in range(2):
                        nc.tensor.matmul(
                            pg[:, r, :], gwt[:, kc, mc * 128:(mc + 1) * 128],
                            mem[:, kc, :], start=(kc == 0), stop=(kc == 1))
                w_t = s4pool.tile([128, 4, 32], BF, tag="w")
                gav = gav_tiles[w]
                nc.vector.tensor_add(
                    w_t[:], gav[:, 0:4, j * 32:(j + 1) * 32], pg[:])
                hh = s4pool.tile([128, 4, 32], BF, tag="hh")
                nc.scalar.activation(hh[:], w_t[:], AF.Relu)
                s4_hh[(w, j)] = (hh, mem)

            def s4B(w, j):
                hh, mem = s4_hh.pop((w, j))
                qg = pQ.tile([128, 4, 32], F32, tag="pq")
                for r in range(4):
                    mc = r % 2
                    goff = 0 if r < 2 else 2
                    gwt = g1w2_t if r < 2 else g2w2_t
                    for kc in range(2):
                        nc.tensor.matmul(
                            qg[:, r, :], gwt[:, kc, mc * 128:(mc + 1) * 128],
                            hh[:, goff + kc, :], start=(kc == 0), stop=(kc == 1))
                gpre = s4pool.tile([128, 4, 32], F32, tag="gpre")
                nc.vector.tensor_add(
                    gpre[:], qg[:],
                    gb2_t[:].unsqueeze(2).broadcast_to([128, 4, 32]))
                gam = s4pool.tile([128, 4, 32], BF, tag="gam")
                nc.scalar.activation(gam[:], gpre[:], AF.Sigmoid)
                m1 = s4pool.tile([128, 2, 32], BF, tag="m1")
                nc.vector.tensor_mul(m1[:], gam[:, 0:2, :], mem[:])
                m2 = s4pool.tile([128, 2, 32], BF, tag="m2")
                chw = chw_tiles[w]
                nc.vector.tensor_mul(
                    m2[:], gam[:, 2:4, :],
                    chw[:].rearrange("p a (t b) -> p a t b", b=32)[:, :, j, :])
                mem_new = mpool.tile([128, 2, 32], BF, tag="mem")
                nc.vector.tensor_add(mem_new[:], m1[:], m2[:])
                mem_cur[0] = mem_new

            # ---------------- prologue ----------------
            h0 = hpool.tile([128, 96], BF, tag="h")
            nc.vector.memset(h0[:], 0.0)
            h_cur[0] = h0
            m0 = mpool.tile([128, 2, 32], BF, tag="mem")
            nc.vector.memset(m0[:], 0.0)
            mem_cur[0] = m0

            dma_x(0)
            if NW > 1:
                dma_x(1)
            # full S1 for window 0 as a warmup burst
            for _ in s1_gen2(0):
                pass

            s1_iters = {}
            s3_iters = {}
            pending_B = [None]

            # ---------------- main superstep loop ----------------
            for w in range(NW + 2):
                if w + 1 < NW:
                    s1_iters[w + 1] = s1_gen2(w + 1)
                if 1 <= w <= NW:
                    s3_iters[w - 1] = s3_gen(w - 1)
                if w + 2 < NW:
                    dma_x(w + 2)
                for j in range(CH):
                    if w < NW:
                        sg, cwt = s2_step(w, j)
                    if pending_B[0] is not None:
                        s4B(*pending_B[0])
                        pending_B[0] = None
                    if w + 1 < NW:
                        next(s1_iters[w + 1], None)
                    if w < NW:
                        s2_tail(w, j, sg, cwt)
                    if 1 <= w <= NW:
                        next(s3_iters[w - 1], None)
                    if 2 <= w <= NW + 1:
                        s4A(w - 2, j)
                        pending_B[0] = (w - 2, j)
            s4B(*pending_B[0])

            # ---------------- output MLP ----------------
            h_fin = h_cur[0]
            mem = mem_cur[0]
            o1p = pQ.tile([128, 4, 32], F32, tag="pq")
            rhs5 = [h_fin[:, 0:32], h_fin[:, 32:64], h_fin[:, 64:96],
                    mem[:, 0, :], mem[:, 1, :]]
            for mc in range(2):
                for kc in range(5):
                    nc.tensor.matmul(
                        o1p[:, mc, :], ow1_t[:, kc, mc * 128:(mc + 1) * 128],
                        rhs5[kc], start=(kc == 0), stop=(kc == 4))
            o1s = s4pool.tile([128, 2, 32], BF, tag="o1s")
            for mc in range(2):
                nc.scalar.activation(o1s[:, mc, :], o1p[:, mc, :], AF.Relu,
                                     bias=ob1_t[:, mc:mc + 1])
            o2p = pE.tile([1, NB], F32, tag="ep")
            for kc in range(2):
                nc.tensor.matmul(o2p[:, 0:32], ow2_t[:, kc, :], o1s[:, kc, :],
                                 start=(kc == 0), stop=(kc == 1))
            o2s = s4pool.tile([1, 32], F32, tag="o2s")
            nc.scalar.activation(o2s[:], o2p[:, 0:32], AF.Identity, bias=ob2_t[:])
            nc.sync.dma_start(out_d.ap().rearrange("b one -> (one) (b)"), o2s[:])

    nc.compile()
    return nc


# ---------------------------------------------------------------------------
# host-side packing
# ---------------------------------------------------------------------------

def pack_shared(inp):
    f = np.float32
    d = {}
    wih = {0: inp["Wih_l"], 1: inp["Wih_a"], 2: inp["Wih_v"]}
    whh = {0: inp["Whh_l"], 1: inp["Whh_a"], 2: inp["Whh_v"]}
    bb = {m: (inp[f"bih_{k}"] + inp[f"bhh_{k}"]).astype(f)
          for m, k in ((0, "l"), (1, "a"), (2, "v"))}
    foff = {0: 0, 1: D_L, 2: D_L + D_A}
    din = {0: D_L, 1: D_A, 2: D_V}

    # row placement in the 512-row packed x / waug (see MM_SPECS)
    ROW = {0: 0, 2: 320, 1: 384}      # modality -> base row
    waug = np.zeros((512, 1536), f)
    whhT = np.zeros((128, 1536), f)
    for gq in range(4):
        tg = TORCH_G[gq]
        for m in range(3):
            s = gq * 3 + m
            wblk = wih[m][tg * 128:(tg + 1) * 128, :]
            r0 = ROW[m]
            waug[r0:r0 + din[m], s * 128:(s + 1) * 128] = wblk.T
            waug[r0 + din[m], s * 128:(s + 1) * 128] = bb[m][tg * 128:(tg + 1) * 128]
            whhT[:, s * 128:(s + 1) * 128] = whh[m][tg * 128:(tg + 1) * 128, :].T
    d["waug"] = waug.astype(NPBF)
    d["whhT"] = whhT.astype(NPBF)
    d["ident"] = np.eye(128, dtype=f).astype(NPBF)
    d["ones128"] = np.ones((128, 1), f).astype(NPBF)

    d["a1w1"] = inp["att1_W1"].T.astype(NPBF).copy()
    d["a1b1c"] = inp["att1_b1"].reshape(2, 128).T.astype(f).copy()
    d["a1w2"] = inp["att1_W2"].T.astype(NPBF).copy()
    d["a1b2c"] = inp["att1_b2"].reshape(6, 128).T.astype(f).copy()
    d["a2w1"] = inp["att2_W1"].T.astype(NPBF).copy()
    d["a2b1c"] = inp["att2_b1"].reshape(2, 128).T.astype(f).copy()
    d["a2w2"] = inp["att2_W2"].T.astype(NPBF).copy()
    d["a2b2c"] = inp["att2_b2"].reshape(2, 128).T.astype(f).copy()
    d["g1a"] = inp["g1_W1"][:, :768].T.astype(NPBF).copy()
    d["g2a"] = inp["g2_W1"][:, :768].T.astype(NPBF).copy()
    d["g1b"] = inp["g1_W1"][:, 768:].T.astype(NPBF).copy()
    d["g2b"] = inp["g2_W1"][:, 768:].T.astype(NPBF).copy()
    d["g1b1c"] = inp["g1_b1"].reshape(2, 128).T.astype(f).copy()
    d["g2b1c"] = inp["g2_b1"].reshape(2, 128).T.astype(f).copy()
    d["g1w2"] = inp["g1_W2"].T.astype(NPBF).copy()
    d["g2w2"] = inp["g2_W2"].T.astype(NPBF).copy()
    d["gb2c"] = np.concatenate([inp["g1_b2"], inp["g2_b2"]]).reshape(4, 128).T.astype(f).copy()
    d["ow1"] = inp["out_W1"].T.astype(NPBF).copy()
    d["ob1"] = inp["out_b1"].reshape(2, 128).T.astype(f).copy()
    d["ow2"] = inp["out_W2"].T.astype(NPBF).copy()
    d["ob2"] = inp["out_b2"].reshape(1, 1).astype(f).copy()
    return d


def pack_x(x, core, Tp):
    """x: [Tp, 256, 409] -> packed [512, Tp*B] (see MM_SPECS row map)."""
    xc = np.asarray(x[:, core * B:(core + 1) * B, :], np.float32)
    xt = xc.transpose(2, 0, 1).reshape(DIN, Tp * B)
    xp = np.zeros((512, Tp * B), np.float32)
    xp[0:300] = xt[0:D_L]                       # l
    xp[300] = 1.0
    xp[320:355] = xt[D_L + D_A:]                # v
    xp[355] = 1.0
    xp[384:458] = xt[D_L:D_L + D_A]             # a
    xp[458] = 1.0
    return xp.astype(NPBF)


_CACHE = {}


def _get_program(Tp):
    if Tp not in _CACHE:
        _CACHE[Tp] = build_program(Tp)
    return _CACHE[Tp]


def kernel(**inputs):
    x = np.asarray(inputs["x"])
    Tp = x.shape[0]
    nc = _get_program(Tp)
    shared = pack_shared({k: np.asarray(v) for k, v in inputs.items()})
    in_maps = []
    for c in range(NCORES):
        m = dict(shared)
        m["xT"] = np.ascontiguousarray(pack_x(x, c, Tp))
        in_maps.append(m)
    res = run_bass_kernel_spmd(nc, in_maps, list(range(NCORES))).results
    out = np.concatenate([r["out"] for r in res], axis=0)
    return out.astype(np.float32)


if __name__ == "__main__":
    import time
    t0 = time.time()
    nc = build_program(64)
    print("built in", time.time() - t0, "s")


# revision 24
# speedup vs baseline: 4.3102x; 1.0904x over previous
"""Trainium2 Bass kernel for nn_Contextual_MFN (Memory Fusion Network).

Fused software-pipelined design (per core; batch DP 8 ways, 32 rows/core).
All phases stream through SBUF in CH=8-step windows with stage skew:

  S1(w+1): xWb = Wih_aug @ x (time-parallel, psum->sbuf)
  S2(w):   3xLSTM recurrence step (gates = inject(xwb) + Whh@h)
  S3(w-1): attention: att1 MLP -> exp -> S -> 1/S -> normalized attended
           -> att2/g1/g2 linear parts (+bias via ones-row matmuls)
  S4(w-2): memory-gate recurrence (mem-dependent matmuls only), split into
           A (L1+relu) and B (L2+sigmoid+mem update) half-steps

One superstep = one (w, j) iteration emitting a slice of every stage, so
the serial chains of S2/S4 hide under S1/S3 tensor work. No barriers, no
intermediate DRAM. All matmuls bf16 (FWL), psum fp32.
"""
import numpy as np
import ml_dtypes

import concourse.bass as bass
import concourse.tile as tile
from concourse import bacc, mybir
from concourse.bass_utils import run_bass_kernel_spmd

F32 = mybir.dt.float32
BF = mybir.dt.bfloat16
AF = mybir.ActivationFunctionType
NPBF = ml_dtypes.bfloat16

T_FULL = 512
NBATCH = 256
NCORES = 8
B = NBATCH // NCORES          # 32 batch rows per core
D_L, D_A, D_V = 300, 74, 35
DIN = D_L + D_A + D_V         # 409
DAUG = DIN + 1                # 410 (ones row for bias)
DH = 128
MEM = 256
CH = 8                        # window size (steps)
NB = CH * B                   # 256: window free dim

TORCH_G = (0, 1, 3, 2)        # our slot g' -> torch gate row block

# x / Waug row packing (512 rows, 4 K-chunks of 128):
#   kc0: l[0:128];  kc1: l[128:256]
#   kc2: rows 0:45 = l[256:300]+bias_l ; rows 64:100 = v[0:35]+bias_v
#   kc3: rows 0:75 = a[0:74]+bias_a
# -> per-slot matmul specs (kc, row_lo, row_hi); partition bases 0/64 only.
MM_SPECS = {
    0: [(0, 0, 128), (1, 0, 128), (2, 0, 45)],   # modality l
    1: [(3, 0, 75)],                             # modality a
    2: [(2, 64, 100)],                           # modality v
}


def build_program(Tp=T_FULL):
    assert Tp % CH == 0
    NW = Tp // CH
    nc = bacc.Bacc("TRN2", target_bir_lowering=False, debug=False)

    # ---------------- external inputs ----------------
    xT = nc.dram_tensor("xT", [512, Tp * B], BF, kind="ExternalInput")
    waug = nc.dram_tensor("waug", [512, 1536], BF, kind="ExternalInput")
    whhT = nc.dram_tensor("whhT", [128, 1536], BF, kind="ExternalInput")
    ident = nc.dram_tensor("ident", [128, 128], BF, kind="ExternalInput")
    ones128 = nc.dram_tensor("ones128", [128, 1], BF, kind="ExternalInput")

    a1w1 = nc.dram_tensor("a1w1", [768, 256], BF, kind="ExternalInput")
    a1b1c = nc.dram_tensor("a1b1c", [128, 2], F32, kind="ExternalInput")
    a1w2 = nc.dram_tensor("a1w2", [256, 768], BF, kind="ExternalInput")
    a1b2c = nc.dram_tensor("a1b2c", [128, 6], F32, kind="ExternalInput")
    a2w1 = nc.dram_tensor("a2w1", [768, 256], BF, kind="ExternalInput")
    a2b1c = nc.dram_tensor("a2b1c", [128, 2], F32, kind="ExternalInput")
    a2w2 = nc.dram_tensor("a2w2", [256, 256], BF, kind="ExternalInput")
    a2b2c = nc.dram_tensor("a2b2c", [128, 2], F32, kind="ExternalInput")
    g1a = nc.dram_tensor("g1a", [768, 256], BF, kind="ExternalInput")
    g2a = nc.dram_tensor("g2a", [768, 256], BF, kind="ExternalInput")
    g1b = nc.dram_tensor("g1b", [256, 256], BF, kind="ExternalInput")
    g2b = nc.dram_tensor("g2b", [256, 256], BF, kind="ExternalInput")
    g1b1c = nc.dram_tensor("g1b1c", [128, 2], F32, kind="ExternalInput")
    g2b1c = nc.dram_tensor("g2b1c", [128, 2], F32, kind="ExternalInput")
    g1w2 = nc.dram_tensor("g1w2", [256, 256], BF, kind="ExternalInput")
    g2w2 = nc.dram_tensor("g2w2", [256, 256], BF, kind="ExternalInput")
    gb2c = nc.dram_tensor("gb2c", [128, 4], F32, kind="ExternalInput")
    ow1 = nc.dram_tensor("ow1", [640, 256], BF, kind="ExternalInput")
    ob1 = nc.dram_tensor("ob1", [128, 2], F32, kind="ExternalInput")
    ow2 = nc.dram_tensor("ow2", [256, 1], BF, kind="ExternalInput")
    ob2 = nc.dram_tensor("ob2", [1, 1], F32, kind="ExternalInput")

    out_d = nc.dram_tensor("out", [B, 1], F32, kind="ExternalOutput")

    import contextlib
    with tile.TileContext(nc) as tc:
        ctx = contextlib.ExitStack()
        with ctx:
            wpool = ctx.enter_context(tc.tile_pool(name="weights", bufs=1))
            hpool = ctx.enter_context(tc.tile_pool(name="hstate", bufs=2))
            xtpool = ctx.enter_context(tc.tile_pool(name="xt", bufs=3))
            xwpool = ctx.enter_context(tc.tile_pool(name="xwb", bufs=3))
            cpool = ctx.enter_context(tc.tile_pool(name="cwin", bufs=3))
            s2pool = ctx.enter_context(tc.tile_pool(name="s2tmp", bufs=3))
            s3pool = ctx.enter_context(tc.tile_pool(name="s3tmp", bufs=2))
            gvpool = ctx.enter_context(tc.tile_pool(name="gav", bufs=2))
            s4pool = ctx.enter_context(tc.tile_pool(name="s4tmp", bufs=3))
            mpool = ctx.enter_context(tc.tile_pool(name="mem", bufs=2))
            # PSUM: exactly 8 banks (pA 2 + pE 3 + pG 1 + pQ 2)
            pA = ctx.enter_context(tc.tile_pool(name="pA", bufs=2, space="PSUM"))
            pE = ctx.enter_context(tc.tile_pool(name="pE", bufs=3, space="PSUM"))
            pG = ctx.enter_context(tc.tile_pool(name="pG", bufs=1, space="PSUM"))
            pQ = ctx.enter_context(tc.tile_pool(name="pQ", bufs=2, space="PSUM"))

            # ---- resident weights / constants ----
            wihT_t = wpool.tile([128, 4, 1536], BF)
            nc.sync.dma_start(wihT_t[:], waug.ap().rearrange("(kc p) c -> p kc c", p=128))
            whhT_t = wpool.tile([128, 1536], BF)
            nc.sync.dma_start(whhT_t[:], whhT.ap())
            id_t = wpool.tile([128, 128], BF)
            nc.sync.dma_start(id_t[:], ident.ap())
            ones128_t = wpool.tile([128, 1], BF)
            nc.sync.dma_start(ones128_t[:], ones128.ap())

            a1w1_t = wpool.tile([128, 6, 256], BF)
            nc.sync.dma_start(a1w1_t[:], a1w1.ap().rearrange("(kc p) c -> p kc c", p=128))
            a1w2_t = wpool.tile([128, 2, 768], BF)
            nc.sync.dma_start(a1w2_t[:], a1w2.ap().rearrange("(kc p) c -> p kc c", p=128))
            a2w1_t = wpool.tile([128, 6, 256], BF)
            nc.sync.dma_start(a2w1_t[:], a2w1.ap().rearrange("(kc p) c -> p kc c", p=128))
            a2w2_t = wpool.tile([128, 2, 256], BF)
            nc.sync.dma_start(a2w2_t[:], a2w2.ap().rearrange("(kc p) c -> p kc c", p=128))
            g1a_t = wpool.tile([128, 6, 256], BF)
            nc.sync.dma_start(g1a_t[:], g1a.ap().rearrange("(kc p) c -> p kc c", p=128))
            g2a_t = wpool.tile([128, 6, 256], BF)
            nc.sync.dma_start(g2a_t[:], g2a.ap().rearrange("(kc p) c -> p kc c", p=128))
            g1b_t = wpool.tile([128, 2, 256], BF)
            nc.sync.dma_start(g1b_t[:], g1b.ap().rearrange("(kc p) c -> p kc c", p=128))
            g2b_t = wpool.tile([128, 2, 256], BF)
            nc.sync.dma_start(g2b_t[:], g2b.ap().rearrange("(kc p) c -> p kc c", p=128))
            g1w2_t = wpool.tile([128, 2, 256], BF)
            nc.sync.dma_start(g1w2_t[:], g1w2.ap().rearrange("(kc p) c -> p kc c", p=128))
            g2w2_t = wpool.tile([128, 2, 256], BF)
            nc.sync.dma_start(g2w2_t[:], g2w2.ap().rearrange("(kc p) c -> p kc c", p=128))
            a1b1c_t = wpool.tile([128, 2], F32)
            nc.sync.dma_start(a1b1c_t[:], a1b1c.ap())
            a1b2c_t = wpool.tile([128, 6], F32)
            nc.sync.dma_start(a1b2c_t[:], a1b2c.ap())
            a2b1c_t = wpool.tile([128, 2], F32)
            nc.sync.dma_start(a2b1c_t[:], a2b1c.ap())
            a2b2c_t = wpool.tile([128, 2], F32)
            nc.sync.dma_start(a2b2c_t[:], a2b2c.ap())
            g1b1c_t = wpool.tile([128, 2], F32)
            nc.sync.dma_start(g1b1c_t[:], g1b1c.ap())
            g2b1c_t = wpool.tile([128, 2], F32)
            nc.sync.dma_start(g2b1c_t[:], g2b1c.ap())
            gb2_t = wpool.tile([128, 4], F32)
            nc.sync.dma_start(gb2_t[:], gb2c.ap())
            ow1_t = wpool.tile([128, 5, 256], BF)
            nc.sync.dma_start(ow1_t[:], ow1.ap().rearrange("(kc p) c -> p kc c", p=128))
            ob1_t = wpool.tile([128, 2], F32)
            nc.sync.dma_start(ob1_t[:], ob1.ap())
            ow2_t = wpool.tile([128, 2, 1], BF)
            nc.sync.dma_start(ow2_t[:], ow2.ap().rearrange("(kc p) c -> p kc c", p=128))
            ob2_t = wpool.tile([1, 1], F32)
            nc.sync.dma_start(ob2_t[:], ob2.ap())

            ones1x128_t = wpool.tile([1, 128], BF)
            nc.vector.memset(ones1x128_t[:], 1.0)
            zero256_t = wpool.tile([128, 256], BF)
            nc.vector.memset(zero256_t[:], 0.0)
            OP = mybir.AluOpType

            # ---------------- per-window state ----------------
            xt_tiles = {}       # w -> x input tile [128, 4, NB]
            xwb_tiles = {}      # w -> xWb tile [128, 12, CH, 32]
            cw_tiles = {}       # w -> c window [128, 3, CH+1, 32]
            gav_tiles = {}      # w -> attended-linear outputs [128, 6, NB]
            chw_tiles = {}      # w -> cHat window [128, 2, NB]
            s4_hh = {}          # (w, j) -> relu tile for S4 B-half

            def dma_x(w):
                t = xtpool.tile([128, 4, NB], BF, tag="xt")
                t0 = w * CH
                for kc in range(4):
                    nc.sync.dma_start(
                        t[:, kc, :],
                        xT.ap()[kc * 128:(kc + 1) * 128, t0 * B:(t0 + CH) * B])
                xt_tiles[w] = t

            def s1_gen2(w):
                """xWb for window w: 6 chunks (one slot-pair each), then 2 empty."""
                xw = xwpool.tile([128, 12, NB], BF, tag="xwb")
                xwb_tiles[w] = xw
                xt = xt_tiles[w]
                for p in range(6):
                    pt = pA.tile([128, 2, NB], F32, tag="big")
                    for si in range(2):
                        s = 2 * p + si
                        specs = MM_SPECS[s % 3]
                        for i, (kc, lo, hi) in enumerate(specs):
                            nc.tensor.matmul(
                                pt[:, si, :], wihT_t[lo:hi, kc, s * 128:(s + 1) * 128],
                                xt[lo:hi, kc, :],
                                start=(i == 0), stop=(i == len(specs) - 1))
                    nc.vector.tensor_copy(xw[:, 2 * p:2 * p + 2, :], pt[:])
                    yield
                for _ in range(2):
                    yield

            h_cur = [None]

            def s2_step(w, j):
                if j == 0:
                    cw = cpool.tile([128, 3, CH + 1, 32], BF, tag="cw")
                    cw_tiles[w] = cw
                    if w == 0:
                        nc.vector.memset(cw[:, :, 0, :], 0.0)
                    else:
                        nc.vector.tensor_copy(cw[:, :, 0, :],
                                              cw_tiles[w - 1][:, :, CH, :])
                cw = cw_tiles[w]
                gp = pG.tile([128, 12, 32], F32, tag="gates")
                nc.tensor.matmul(gp[:], id_t[:],
                                 xwb_tiles[w][:, :, j * 32:(j + 1) * 32],
                                 start=True, stop=False)
                h = h_cur[0]
                for s in range(12):
                    gq, m = divmod(s, 3)
                    nc.tensor.matmul(
                        gp[:, s, :], whhT_t[:, s * 128:(s + 1) * 128],
                        h[:, m * 32:(m + 1) * 32],
                        start=False, stop=(s == 11))
                sg = s2pool.tile([128, 9, 32], BF, tag="sg")
                nc.scalar.activation(sg[:], gp[:, 0:9, :], AF.Sigmoid)
                tg = s2pool.tile([128, 3, 32], BF, tag="tg")
                nc.scalar.activation(tg[:], gp[:, 9:12, :], AF.Tanh)
                t1 = s2pool.tile([128, 3, 32], BF, tag="t1")
                nc.vector.tensor_mul(t1[:], sg[:, 0:3, :], tg[:])
                t2 = s2pool.tile([128, 3, 32], BF, tag="t2")
                nc.vector.tensor_mul(t2[:], sg[:, 3:6, :], cw[:, :, j, :])
                nc.vector.tensor_add(cw[:, :, j + 1, :], t1[:], t2[:])
                return sg, cw

            def s2_tail(w, j, sg, cw):
                tct = s2pool.tile([128, 3, 32], BF, tag="tc")
                nc.scalar.activation(tct[:], cw[:, :, j + 1, :], AF.Tanh)
                h_new = hpool.tile([128, 96], BF, tag="h")
                nc.vector.tensor_mul(
                    h_new[:].rearrange("p (m b) -> p m b", b=32),
                    sg[:, 6:9, :], tct[:])
                h_cur[0] = h_new

            def s3_gen(w):
                """Attention for window w, 8 chunks."""
                cw = cw_tiles[w]

                def rhs_k(kc):
                    if kc < 3:
                        return cw[:, kc, 0:CH, :]
                    return cw[:, kc - 3, 1:CH + 1, :]

                # --- chunk 0: att1 L1; relu+bias via DVE stt ---
                y1p = pA.tile([128, 2, NB], F32, tag="big")
                for mc in range(2):
                    for kc in range(6):
                        nc.tensor.matmul(
                            y1p[:, mc, :], a1w1_t[:, kc, mc * 128:(mc + 1) * 128],
                            rhs_k(kc), start=(kc == 0), stop=(kc == 5))
                y1 = s3pool.tile([128, 2, NB], BF, tag="y1")
                for mc in range(2):
                    nc.vector.scalar_tensor_tensor(
                        y1[:, mc, :], y1p[:, mc, :], a1b1c_t[:, mc:mc + 1],
                        zero256_t[:], OP.add, OP.max)
                yield
                # --- chunks 1-3: att1 L2 pairs (exps clustered in chunk 3) ---
                et = s3pool.tile([128, 6, NB], BF, tag="et")
                eps = []
                for pr in range(3):
                    ep = pE.tile([128, 2, NB], F32, tag="ep")
                    eps.append(ep)
                    for si in range(2):
                        mc6 = 2 * pr + si
                        for kc in range(2):
                            nc.tensor.matmul(
                                ep[:, si, :], a1w2_t[:, kc, mc6 * 128:(mc6 + 1) * 128],
                                y1[:, kc, :], start=(kc == 0), stop=(kc == 1))
                    if pr == 2:
                        # atomic cluster: one table switch in, one out.
                        # exp4 (latest-ready) leads so the unit starts only
                        # when every ep is done.
                        with tc.tile_critical():
                            for q in (4, 5, 0, 1, 2, 3):
                                nc.scalar.activation(et[:, q, :],
                                                     eps[q // 2][:, q % 2, :],
                                                     AF.Exp, bias=a1b2c_t[:, q:q + 1])
                    yield
                # --- chunk 4: S sums, 1/S ---
                sp = pE.tile([1, NB], F32, tag="ep")
                for q in range(6):
                    nc.tensor.matmul(sp[:], ones128_t[:], et[:, q, :],
                                     start=(q == 0), stop=(q == 5),
                                     skip_group_check=True)
                sinv = s3pool.tile([1, NB], F32, tag="sinvf")
                nc.vector.reciprocal_approx_fast(sinv[:], sp[:])
                sinvb = s3pool.tile([1, NB], BF, tag="sinvb")
                nc.vector.tensor_copy(sinvb[:], sinv[:])
                yield
                # --- chunk 5: broadcast, normalized cStar, utn; att2 L1 ---
                sbp = pA.tile([128, 2, NB], F32, tag="big")
                nc.tensor.matmul(sbp[:, 0, :], ones1x128_t[:], sinvb[:],
                                 start=True, stop=True)
                sb_w = s3pool.tile([128, NB], BF, tag="sbw")
                nc.vector.tensor_copy(sb_w[:], sbp[:, 0, :])
                cn = s3pool.tile([128, 6, NB], BF, tag="cn")
                sbv = sb_w[:].rearrange("p (t b) -> p t b", b=32)
                nc.vector.tensor_mul(
                    cn[:, 0:3, :].rearrange("p m (t b) -> p m t b", b=32),
                    cw[:, :, 0:CH, :],
                    sbv.unsqueeze(1).broadcast_to([128, 3, CH, 32]))
                nc.vector.tensor_mul(
                    cn[:, 3:6, :].rearrange("p m (t b) -> p m t b", b=32),
                    cw[:, :, 1:CH + 1, :],
                    sbv.unsqueeze(1).broadcast_to([128, 3, CH, 32]))
                utn = s3pool.tile([128, 6, NB], BF, tag="utn")
                nc.vector.tensor_mul(utn[:], et[:], cn[:])
                zp = pA.tile([128, 2, NB], F32, tag="big")
                for mc in range(2):
                    for kc in range(6):
                        nc.tensor.matmul(
                            zp[:, mc, :], a2w1_t[:, kc, mc * 128:(mc + 1) * 128],
                            utn[:, kc, :], start=(kc == 0), stop=(kc == 5))
                z = s3pool.tile([128, 2, NB], BF, tag="z")
                for mc in range(2):
                    nc.vector.scalar_tensor_tensor(
                        z[:, mc, :], zp[:, mc, :], a2b1c_t[:, mc:mc + 1],
                        zero256_t[:], OP.add, OP.max)
                yield
                # --- chunk 6: att2 L2 + cHat tanh(x+b2); g1 (+b1 via stt) ---
                gav = gvpool.tile([128, 6, NB], BF, tag="gav")
                gav_tiles[w] = gav
                ap2 = pA.tile([128, 2, NB], F32, tag="big")
                for mc in range(2):
                    for kc in range(2):
                        nc.tensor.matmul(
                            ap2[:, mc, :], a2w2_t[:, kc, mc * 128:(mc + 1) * 128],
                            z[:, kc, :], start=(kc == 0), stop=(kc == 1))
                chw = gvpool.tile([128, 2, NB], BF, tag="chw")
                chw_tiles[w] = chw
                for mc in range(2):
                    nc.scalar.activation(chw[:, mc, :], ap2[:, mc, :], AF.Tanh,
                                         bias=a2b2c_t[:, mc:mc + 1])
                g1p = pA.tile([128, 2, NB], F32, tag="big")
                for mc in range(2):
                    for kc in range(6):
                        nc.tensor.matmul(
                            g1p[:, mc, :], g1a_t[:, kc, mc * 128:(mc + 1) * 128],
                            utn[:, kc, :], start=(kc == 0), stop=(kc == 5))
                for mc in range(2):
                    nc.vector.scalar_tensor_tensor(
                        gav[:, mc, :], g1p[:, mc, :], g1b1c_t[:, mc:mc + 1],
                        zero256_t[:], OP.add, OP.add)
                yield
                # --- chunk 7: g2 (+b1 via stt) ---
                g2p = pA.tile([128, 2, NB], F32, tag="big")
                for mc in range(2):
                    for kc in range(6):
                        nc.tensor.matmul(
                            g2p[:, mc, :], g2a_t[:, kc, mc * 128:(mc + 1) * 128],
                            utn[:, kc, :], start=(kc == 0), stop=(kc == 5))
                for mc in range(2):
                    nc.vector.scalar_tensor_tensor(
                        gav[:, 2 + mc, :], g2p[:, mc, :], g2b1c_t[:, mc:mc + 1],
                        zero256_t[:], OP.add, OP.add)
                yield

            mem_cur = [None]

            def s4A(w, j):
                pg = pQ.tile([128, 4, 32], F32, tag="pq")
                mem = mem_cur[0]
                for r in range(4):
                    mc = r % 2
                    gwt = g1b_t if r < 2 else g2b_t
                    for kc in range(2):
                        nc.tensor.matmul(
                            pg[:, r, :], gwt[:, kc, mc * 128:(mc + 1) * 128],
                            mem[:, kc, :], start=(kc == 0), stop=(kc == 1))
                w_t = s4pool.tile([128, 4, 32], BF, tag="w")
                gav = gav_tiles[w]
                nc.vector.tensor_add(
                    w_t[:], gav[:, 0:4, j * 32:(j + 1) * 32], pg[:])
                hh = s4pool.tile([128, 4, 32], BF, tag="hh")
                nc.scalar.activation(hh[:], w_t[:], AF.Relu)
                s4_hh[(w, j)] = (hh, mem)

            def s4B(w, j):
                hh, mem = s4_hh.pop((w, j))
                qg = pQ.tile([128, 4, 32], F32, tag="pq")
                for r in range(4):
                    mc = r % 2
                    goff = 0 if r < 2 else 2
                    gwt = g1w2_t if r < 2 else g2w2_t
                    for kc in range(2):
                        nc.tensor.matmul(
                            qg[:, r, :], gwt[:, kc, mc * 128:(mc + 1) * 128],
                            hh[:, goff + kc, :], start=(kc == 0), stop=(kc == 1))
                gpre = s4pool.tile([128, 4, 32], F32, tag="gpre")
                nc.vector.tensor_add(
                    gpre[:], qg[:],
                    gb2_t[:].unsqueeze(2).broadcast_to([128, 4, 32]))
                gam = s4pool.tile([128, 4, 32], BF, tag="gam")
                nc.scalar.activation(gam[:], gpre[:], AF.Sigmoid)
                m1 = s4pool.tile([128, 2, 32], BF, tag="m1")
                nc.vector.tensor_mul(m1[:], gam[:, 0:2, :], mem[:])
                m2 = s4pool.tile([128, 2, 32], BF, tag="m2")
                chw = chw_tiles[w]
                nc.vector.tensor_mul(
                    m2[:], gam[:, 2:4, :],
                    chw[:].rearrange("p a (t b) -> p a t b", b=32)[:, :, j, :])
                mem_new = mpool.tile([128, 2, 32], BF, tag="mem")
                nc.vector.tensor_add(mem_new[:], m1[:], m2[:])
                mem_cur[0] = mem_new

            # ---------------- prologue ----------------
            h0 = hpool.tile([128, 96], BF, tag="h")
            nc.vector.memset(h0[:], 0.0)
            h_cur[0] = h0
            m0 = mpool.tile([128, 2, 32], BF, tag="mem")
            nc.vector.memset(m0[:], 0.0)
            mem_cur[0] = m0

            dma_x(0)
            if NW > 1:
                dma_x(1)
            # full S1 for window 0 as a warmup burst
            for _ in s1_gen2(0):
                pass

            s1_iters = {}
            s3_iters = {}
            pending_B = [None]

            # ---------------- main superstep loop ----------------
            for w in range(NW + 2):
                if w + 1 < NW:
                    s1_iters[w + 1] = s1_gen2(w + 1)
                if 1 <= w <= NW:
                    s3_iters[w - 1] = s3_gen(w - 1)
                if w + 2 < NW:
                    dma_x(w + 2)
                for j in range(CH):
                    if w < NW:
                        sg, cwt = s2_step(w, j)
                    if pending_B[0] is not None:
                        s4B(*pending_B[0])
                        pending_B[0] = None
                    if w + 1 < NW:
                        next(s1_iters[w + 1], None)
                    if w < NW:
                        s2_tail(w, j, sg, cwt)
                    if 1 <= w <= NW:
                        next(s3_iters[w - 1], None)
                    if 2 <= w <= NW + 1:
                        s4A(w - 2, j)
                        pending_B[0] = (w - 2, j)
            s4B(*pending_B[0])

            # ---------------- output MLP ----------------
            h_fin = h_cur[0]
            mem = mem_cur[0]
            o1p = pQ.tile([128, 4, 32], F32, tag="pq")
            rhs5 = [h_fin[:, 0:32], h_fin[:, 32:64], h_fin[:, 64:96],
                    mem[:, 0, :], mem[:, 1, :]]
            for mc in range(2):
                for kc in range(5):
                    nc.tensor.matmul(
                        o1p[:, mc, :], ow1_t[:, kc, mc * 128:(mc + 1) * 128],
                        rhs5[kc], start=(kc == 0), stop=(kc == 4))
            o1s = s4pool.tile([128, 2, 32], BF, tag="o1s")
            for mc in range(2):
                nc.scalar.activation(o1s[:, mc, :], o1p[:, mc, :], AF.Relu,
                                     bias=ob1_t[:, mc:mc + 1])
            o2p = pE.tile([1, NB], F32, tag="ep")
            for kc in range(2):
                nc.tensor.matmul(o2p[:, 0:32], ow2_t[:, kc, :], o1s[:, kc, :],
                                 start=(kc == 0), stop=(kc == 1))
            o2s = s4pool.tile([1, 32], F32, tag="o2s")
            nc.scalar.activation(o2s[:], o2p[:, 0:32], AF.Identity, bias=ob2_t[:])
            nc.sync.dma_start(out_d.ap().rearrange("b one -> (one) (b)"), o2s[:])

    nc.compile()
    return nc


# ---------------------------------------------------------------------------
# host-side packing
# ---------------------------------------------------------------------------

def pack_shared(inp):
    f = np.float32
    d = {}
    wih = {0: inp["Wih_l"], 1: inp["Wih_a"], 2: inp["Wih_v"]}
    whh = {0: inp["Whh_l"], 1: inp["Whh_a"], 2: inp["Whh_v"]}
    bb = {m: (inp[f"bih_{k}"] + inp[f"bhh_{k}"]).astype(f)
          for m, k in ((0, "l"), (1, "a"), (2, "v"))}
    foff = {0: 0, 1: D_L, 2: D_L + D_A}
    din = {0: D_L, 1: D_A, 2: D_V}

    # row placement in the 512-row packed x / waug (see MM_SPECS)
    ROW = {0: 0, 2: 320, 1: 384}      # modality -> base row
    waug = np.zeros((512, 1536), f)
    whhT = np.zeros((128, 1536), f)
    for gq in range(4):
        tg = TORCH_G[gq]
        for m in range(3):
            s = gq * 3 + m
            wblk = wih[m][tg * 128:(tg + 1) * 128, :]
            r0 = ROW[m]
            waug[r0:r0 + din[m], s * 128:(s + 1) * 128] = wblk.T
            waug[r0 + din[m], s * 128:(s + 1) * 128] = bb[m][tg * 128:(tg + 1) * 128]
            whhT[:, s * 128:(s + 1) * 128] = whh[m][tg * 128:(tg + 1) * 128, :].T
    d["waug"] = waug.astype(NPBF)
    d["whhT"] = whhT.astype(NPBF)
    d["ident"] = np.eye(128, dtype=f).astype(NPBF)
    d["ones128"] = np.ones((128, 1), f).astype(NPBF)

    d["a1w1"] = inp["att1_W1"].T.astype(NPBF).copy()
    d["a1b1c"] = inp["att1_b1"].reshape(2, 128).T.astype(f).copy()
    d["a1w2"] = inp["att1_W2"].T.astype(NPBF).copy()
    d["a1b2c"] = inp["att1_b2"].reshape(6, 128).T.astype(f).copy()
    d["a2w1"] = inp["att2_W1"].T.astype(NPBF).copy()
    d["a2b1c"] = inp["att2_b1"].reshape(2, 128).T.astype(f).copy()
    d["a2w2"] = inp["att2_W2"].T.astype(NPBF).copy()
    d["a2b2c"] = inp["att2_b2"].reshape(2, 128).T.astype(f).copy()
    d["g1a"] = inp["g1_W1"][:, :768].T.astype(NPBF).copy()
    d["g2a"] = inp["g2_W1"][:, :768].T.astype(NPBF).copy()
    d["g1b"] = inp["g1_W1"][:, 768:].T.astype(NPBF).copy()
    d["g2b"] = inp["g2_W1"][:, 768:].T.astype(NPBF).copy()
    d["g1b1c"] = inp["g1_b1"].reshape(2, 128).T.astype(f).copy()
    d["g2b1c"] = inp["g2_b1"].reshape(2, 128).T.astype(f).copy()
    d["g1w2"] = inp["g1_W2"].T.astype(NPBF).copy()
    d["g2w2"] = inp["g2_W2"].T.astype(NPBF).copy()
    d["gb2c"] = np.concatenate([inp["g1_b2"], inp["g2_b2"]]).reshape(4, 128).T.astype(f).copy()
    d["ow1"] = inp["out_W1"].T.astype(NPBF).copy()
    d["ob1"] = inp["out_b1"].reshape(2, 128).T.astype(f).copy()
    d["ow2"] = inp["out_W2"].T.astype(NPBF).copy()
    d["ob2"] = inp["out_b2"].reshape(1, 1).astype(f).copy()
    return d


def pack_x(x, core, Tp):
    """x: [Tp, 256, 409] -> packed [512, Tp*B] (see MM_SPECS row map)."""
    xc = np.asarray(x[:, core * B:(core + 1) * B, :], np.float32)
    xt = xc.transpose(2, 0, 1).reshape(DIN, Tp * B)
    xp = np.zeros((512, Tp * B), np.float32)
    xp[0:300] = xt[0:D_L]                       # l
    xp[300] = 1.0
    xp[320:355] = xt[D_L + D_A:]                # v
    xp[355] = 1.0
    xp[384:458] = xt[D_L:D_L + D_A]             # a
    xp[458] = 1.0
    return xp.astype(NPBF)


_CACHE = {}


def _get_program(Tp):
    if Tp not in _CACHE:
        _CACHE[Tp] = build_program(Tp)
    return _CACHE[Tp]


def kernel(**inputs):
    x = np.asarray(inputs["x"])
    Tp = x.shape[0]
    nc = _get_program(Tp)
    shared = pack_shared({k: np.asarray(v) for k, v in inputs.items()})
    in_maps = []
    for c in range(NCORES):
        m = dict(shared)
        m["xT"] = np.ascontiguousarray(pack_x(x, c, Tp))
        in_maps.append(m)
    res = run_bass_kernel_spmd(nc, in_maps, list(range(NCORES))).results
    out = np.concatenate([r["out"] for r in res], axis=0)
    return out.astype(np.float32)


if __name__ == "__main__":
    import time
    t0 = time.time()
    nc = build_program(64)
    print("built in", time.time() - t0, "s")
